# revision 55
# baseline (speedup 1.0000x reference)
"""Trainium2 Bass kernel for nn_AttentionApproximator (sparse_attention).

Math (per batch b):
  scores = relu(full @ sw1 + sb1) @ sw2 + sb2            [S]
  top_idx = top_k(scores, k=204)                          (set only matters)
  sel     = full[top_idx]                                 [k, d]
  q_part  = full @ mw1[:d]                                [S, 64]
  kvb     = sel @ (mw1[d:2d] + mw1[2d:]) + mb1            [k, 64]
  h1      = relu(q_part[s] + kvb[j])                      [S, k, 64]
  h2      = relu(h1 @ mw2 + mb2)                          [S, k, 32]
  out     = mean_j(h2) @ mw3 + mb3                        [S, d]

Device strategy (8 cores, SPMD): core c handles batch b=c//2, query rows
h=c%2 (1024 of 2048).  Top-k via exact ranks (rank_i = #{j: s_j > s_i});
rank doubles as the compaction slot, gathered by one-hot matmul.

Perf structure vs the original version:
  - every stage matmul runs in bf16 (1 PE cycle/row instead of 4);
    scores stay self-consistent (fp32 PSUM accum, exact transposes).
  - h1 tiles are produced bf16 from a bf16 qT2 -> DVE 4x mode (327ns/op).
  - h2 is written as fp8 into paired super-tiles; the mw3 stage is a
    single fp8 DoubleRow matmul (half PE rate) whose stationary carries
    q8(mw3) in out-rows 0:16 and the quantization residual in 16:32;
    the host adds the residual rows after DMA.
  - the kv selection is fused: G = fb^T @ onehot gathers raw token
    features on the PE, then one wkv matmul yields the slot-pair kvb
    (no per-chunk kv matmuls or PSUM->SBUF copies).
  - scores broadcast via gpsimd partition_broadcast (the only legal
    Pool op here: the GPSIMD ISA has no tensor_scalar and cannot read
    PSUM); all other elementwise work is balanced across DVE and ACT.
  - h1 producers are emitted LOOK groups ahead so the PE stream never
    waits on its h1 inputs; engine splits are env-tunable knobs.
"""

import os
from contextlib import ExitStack

import numpy as np

B, S, D = 4, 2048, 16
DA = D + 1               # augmented with ones row
K = 204                  # top-k  (int(2048*0.1))
KP = K // 2              # 102 slot-pairs
H1 = 64
H2 = 32
SH = S // 2              # 1024 query rows per core
NCH = S // 128           # 16 token chunks
NG = KP // 2             # 51 groups of 4 tokens
N_CORES = 8
INV_K = float(np.float32(1.0) / np.float32(K))

# ---- tuning knobs ----
def _env(name, dflt):
    return int(os.environ.get(name, str(dflt)))

Z8Q = _env("KERNEL_Z8Q", 0)          # groups on the fp8-DoubleRow mw2 path
H18_ACT = _env("KERNEL_H18_ACT", 0)  # fp8 h1 halves made by ACT (rest DVE)
M_ACT = _env("KERNEL_M_ACT", 44)     # h2 jobs on ACT (of 51, rest DVE)
M_TAILD = _env("KERNEL_M_TAILD", 0)  # last groups' h2 forced to DVE
B_ACT = _env("KERNEL_B_ACT", 6)      # rank chunks on ACT (of 16, rest DVE)
H1_ACT = _env("KERNEL_H1_ACT", 0)    # bf16 h1 pairs on ACT (of 102, rest DVE)
LOOK = _env("KERNEL_LOOK", 3)        # h1 producer lookahead (groups)

# const-tile column layout (bf16 tensor)
C16_IOTA = 0             # [128, 204] one-hot iota (evens 0:102, odds 102:204)
C16_SW1 = 204            # [DA, 32]
C16_SW2 = 236            # [H2, 1]
C16_WKV = 237            # [DA, 64]
C16_WQ = 301             # [D, 64]
C16_BDMW2 = 365          # [128, 64]
C16_W = 429
# fp32 const tensor
C32_MB24 = 0             # [128, 1] mb2 tiled x4
C32_MB3X = 1             # [128, 1] rows 0:16 = mb3, else 0
C32_ONE = 2              # [1, 1] = 1.0 (transpose aux)
C32_WARM = 16            # [1, 512] zeros (PE warmup)
C32_W = 528

_cache = {}


def _spread(n, total):
    """n indices spread evenly over range(total)."""
    if n <= 0:
        return []
    return sorted({min(total - 1, int(round(i * total / n))) for i in range(n)})


def _build_module():
    import concourse.mybir as mybir
    import concourse.tile as tile
    from concourse import bacc

    fp32 = mybir.dt.float32
    bf16 = mybir.dt.bfloat16
    f8 = mybir.dt.float8e4
    Alu = mybir.AluOpType
    Act = mybir.ActivationFunctionType
    DR = mybir.MatmulPerfMode.DoubleRow

    nc = bacc.Bacc("TRN2", target_bir_lowering=False, debug=False,
                   num_devices=N_CORES)

    d_fbT16 = nc.dram_tensor("fbT16", [DA, S], bf16, kind="ExternalInput").ap()
    d_fb16 = nc.dram_tensor("fb16", [128, DA * NCH], bf16,
                            kind="ExternalInput").ap()
    d_fqT16 = nc.dram_tensor("fqT16", [D, SH], bf16, kind="ExternalInput").ap()
    d_c16 = nc.dram_tensor("c16", [128, C16_W], bf16, kind="ExternalInput").ap()
    d_c32 = nc.dram_tensor("c32", [128, C32_W], fp32, kind="ExternalInput").ap()
    d_c8 = nc.dram_tensor("c8", [128, 2, 256], f8, kind="ExternalInput").ap()
    d_outT = nc.dram_tensor("outT", [2 * D, SH], fp32,
                            kind="ExternalOutput").ap()

    fp8_groups = set(_spread(Z8Q, NG))
    # engine per fp8-h1 half (2 halves per fp8 group)
    n8 = 2 * len(fp8_groups)
    eng8 = ["dve"] * n8
    for i in _spread(min(H18_ACT, n8), n8):
        eng8[i] = "act"
    # engine per h2 job (group): ACT or DVE (Pool cannot read PSUM, and
    # the GPSIMD ISA has no tensor_scalar)
    h2_eng = ["dve"] * NG
    for g in _spread(M_ACT, NG):
        h2_eng[g] = "act"
    for g in range(NG - M_TAILD, NG):
        h2_eng[g] = "dve"     # DVE drains first; keep ACT off the tail
    # engine per rank chunk
    b_eng = ["dve"] * NCH
    for c in _spread(B_ACT, NCH):
        b_eng[c] = "act"
    # bf16 h1 pairs lifted to ACT to balance DVE
    h1_eng = ["dve"] * KP
    for p in _spread(H1_ACT, KP):
        h1_eng[p] = "act"

    with tile.TileContext(nc) as tc:
        with (
            ExitStack() as es,
            tc.tile_pool(name="const", bufs=1) as cpool,
            tc.tile_pool(name="sel", bufs=1) as spool,
            tc.tile_pool(name="scratch", bufs=2) as scpool,
            tc.tile_pool(name="h1p", bufs=2 * (LOOK + 2)) as h1pool,
            tc.tile_pool(name="h18p", bufs=LOOK + 2) as h18pool,
            tc.tile_pool(name="h2p", bufs=3) as h2pool,
        ):
            # ---- constants / inputs to SBUF (critical-path DMAs first) ----
            sb_c16 = cpool.tile([128, C16_W], bf16)
            nc.sync.dma_start(sb_c16[:], d_c16[:])
            sb_fbT16 = cpool.tile([DA, S], bf16)
            nc.sync.dma_start(sb_fbT16[:, 0:1024], d_fbT16[:, 0:1024])
            nc.sync.dma_start(sb_fbT16[:, 1024:2048], d_fbT16[:, 1024:2048])
            sb_fqT16 = cpool.tile([D, SH], bf16)
            nc.sync.dma_start(sb_fqT16[:], d_fqT16[:])
            sb_fb16 = cpool.tile([128, DA * NCH], bf16)
            nc.sync.dma_start(sb_fb16[:], d_fb16[:])
            sb_c32 = cpool.tile([128, C32_W], fp32)
            nc.sync.dma_start(sb_c32[:], d_c32[:])
            sb_c8 = cpool.tile([128, 2, 256], f8)
            nc.sync.dma_start(sb_c8[:], d_c8[:])

            sb_iota = sb_c16[:, C16_IOTA:C16_IOTA + K]
            sb_sw1 = sb_c16[0:DA, C16_SW1:C16_SW1 + H2]
            sb_sw2 = sb_c16[0:H2, C16_SW2:C16_SW2 + 1]
            sb_wkv = sb_c16[0:DA, C16_WKV:C16_WKV + H1]
            sb_wq = sb_c16[0:D, C16_WQ:C16_WQ + H1]
            sb_bdmw2 = sb_c16[:, C16_BDMW2:C16_BDMW2 + H1]
            sb_mb24 = sb_c32[:, C32_MB24:C32_MB24 + 1]
            sb_mb3x = sb_c32[:, C32_MB3X:C32_MB3X + 1]
            sb_one = sb_c32[0:1, C32_ONE:C32_ONE + 1]
            sb_w28 = sb_c8[:, :, 0:128]
            sb_w348 = sb_c8[:, :, 128:256]

            # ---- PE warmup: ramps the PE clock while input DMAs stream ----
            with tc.tile_pool(name="pswarm", bufs=1, space="PSUM") as pW:
                ps_w = pW.tile([1, 429], fp32)
                nc.tensor.matmul(ps_w[0:1, :], sb_c16[0:1, 0:1],
                                 sb_c16[0:1, 0:429],
                                 start=True, stop=True)

            # ---- stage A: score hidden layer + per-token scores ----
            # pbcast chunks are emitted right behind each score-row copy so
            # the Pool engine streams the broadcast while stage A finishes.
            sb_srs = []
            sb_bc = cpool.tile([128, S], fp32)
            with tc.tile_pool(name="psA", bufs=1, space="PSUM") as pA, \
                 tc.tile_pool(name="psA15", bufs=1, space="PSUM") as pA15:
                sb_Hs = []
                for n in range(4):
                    sl = slice(n * 512, (n + 1) * 512)
                    ps_Hn = pA.tile([H2, 512], fp32, tag=f"H{n}")
                    nc.tensor.matmul(ps_Hn[:], sb_sw1, sb_fbT16[:, sl],
                                     start=True, stop=True)
                    sb_Hn = spool.tile([H2, 512], bf16, tag=f"sbH{n}")
                    if n % 2 == 0:     # alternate engines: shorter serial chain
                        nc.scalar.activation(sb_Hn[:], ps_Hn[:], Act.Relu)
                    else:
                        nc.vector.tensor_scalar(sb_Hn[:], ps_Hn[:], 0.0, None,
                                                Alu.max)
                    sb_Hs.append(sb_Hn)
                # scores: fp32 PSUM accumulation of bf16 products; all
                # downstream layouts derive from these rows exactly.
                for n in range(4):
                    sl = slice(n * 512, (n + 1) * 512)
                    ps_srn = pA15.tile([1, 512], fp32, tag=f"sr{n}")
                    nc.tensor.matmul(ps_srn[:], sb_sw2, sb_Hs[n][:],
                                     start=True, stop=True)
                    sb_srn = spool.tile([1, 512], fp32, tag=f"sbsr{n}")
                    nc.scalar.copy(sb_srn[:], ps_srn[:])
                    sb_srs.append(sb_srn)
                    nc.gpsimd.partition_broadcast(sb_bc[:, sl], sb_srn[0:1, :])

            with tc.tile_pool(name="psA2", bufs=1, space="PSUM") as pA2:
                # scores token-major via PE transposes (exact), in 4-column
                # tiles so early rank chunks start before stage A drains
                sb_stoks, sb_nstoks = [], []
                for t in range(4):
                    ps_st = pA2.tile([128, 4], fp32, tag=f"pst{t}")
                    for i in range(4):
                        nc.tensor.transpose(
                            ps_st[:, i:i + 1],
                            sb_srs[t][0:1, i * 128:(i + 1) * 128],
                            sb_one)
                    sb_st = spool.tile([128, 4], fp32, tag=f"stok{t}")
                    nc.vector.tensor_copy(sb_st[:], ps_st[:])
                    sb_stoks.append(sb_st)
                    sb_nst = spool.tile([128, 4], fp32, tag=f"nstok{t}")
                    nc.vector.tensor_scalar(sb_nst[:], sb_st[:], -1.0, None,
                                            Alu.mult)
                    sb_nstoks.append(sb_nst)

                def stok_col(c):
                    return sb_stoks[c // 4][:, c % 4:c % 4 + 1]

                def nstok_col(c):
                    return sb_nstoks[c // 4][:, c % 4:c % 4 + 1]

            es_c = ExitStack()
            pCs = es_c.enter_context(tc.tile_pool(name="psCsel", bufs=1,
                                                  space="PSUM"))

            # ---- stage D matmuls (qT2 copy is emitted after stage B so it
            # doesn't block ACT's rank chunks) ----
            pD = es_c.enter_context(tc.tile_pool(name="psD", bufs=1,
                                                 space="PSUM"))
            ps_q = pD.tile([128, SH], fp32)
            for half in range(2):
                rows = slice(half * H1, (half + 1) * H1)
                for n in range(2):
                    sl = slice(n * 512, (n + 1) * 512)
                    nc.tensor.matmul(ps_q[rows, sl], sb_wq,
                                     sb_fqT16[:, sl], start=True,
                                     stop=True)

            # ---- stage B: exact ranks (token-major); the raw-feature
            # gather G = fb^T . onehot is emitted per-chunk so it streams
            # behind the ranks ----
            ps_G = pCs.tile([17, 256], fp32)   # 1 bank (uses [:, 0:K])

            def rank_chunk(c):
                rk = spool.tile([128, 1], fp32, tag=f"rank{c}")
                if b_eng[c] == "act":
                    scr = scpool.tile([128, S], fp32, tag="cmp_scr2")
                    rsgn = spool.tile([128, 1], fp32, tag=f"rsgn{c}")
                    nc.scalar.activation(scr[:], sb_bc[:], Act.Sign,
                                         bias=nstok_col(c),
                                         accum_out=rsgn[:])
                    nc.vector.tensor_scalar(rk[:], rsgn[:], 0.5,
                                            1023.5, Alu.mult, Alu.add)
                else:
                    scr = scpool.tile([128, S], fp32, tag="cmp_scr")
                    nc.vector.tensor_scalar(
                        scr[:], sb_bc[:], stok_col(c), 0.0,
                        Alu.is_gt, Alu.add, accum_out=rk[:])
                return rk

            for c in range(NCH):
                rk = rank_chunk(c)
                oh = scpool.tile([128, K], bf16, tag="oh")
                nc.vector.tensor_scalar(oh[:], sb_iota, rk[:], None,
                                        Alu.is_equal)
                nc.tensor.matmul(ps_G[:, 0:K],
                                 sb_fb16[:, c * DA:(c + 1) * DA], oh[:],
                                 start=(c == 0), stop=(c == NCH - 1))

            sb_qT2 = cpool.tile([128, SH], bf16)
            nc.scalar.copy(sb_qT2[:], ps_q[:])
            # selected raw features -> SBUF, then kvb for slot pairs via wkv:
            # evens land in rows 0:64, odds in rows 64:128 (strided moving AP)
            sb_G = spool.tile([17, K], bf16)
            nc.scalar.copy(sb_G[:], ps_G[:, 0:K])
            pK = es_c.enter_context(tc.tile_pool(name="psKvb", bufs=1,
                                                 space="PSUM"))
            ps_kvb = pK.tile([128, KP], fp32)
            nc.tensor.matmul(ps_kvb[0:H1, :], sb_wkv, sb_G[:, 0:KP],
                             start=True, stop=True)
            nc.tensor.matmul(ps_kvb[H1:128, :], sb_wkv, sb_G[:, KP:K],
                             start=True, stop=True)
            sb_kvb2a = spool.tile([128, 51], fp32)
            nc.scalar.copy(sb_kvb2a[:], ps_kvb[:, 0:51])
            sb_kvb2b = spool.tile([128, KP - 51], fp32)
            nc.scalar.copy(sb_kvb2b[:], ps_kvb[:, 51:KP])
            es_c.close()

            def kvb_col(p):
                return (sb_kvb2a[:, p:p + 1] if p < 51
                        else sb_kvb2b[:, p - 51:p - 50])

            pmain = es.enter_context(
                tc.tile_pool(name="main_psum", bufs=3, space="PSUM"))
            pout = es.enter_context(
                tc.tile_pool(name="out_psum", bufs=1, space="PSUM"))

            # ---- stage E: main pairwise loop over 51 groups of 4 tokens.
            # h1 producers are emitted LOOK groups ahead of their matmuls so
            # slow ACT/Pool h1 tiles never stall the PE stream. ----
            ps_out = pout.tile([128, SH], fp32, tag="outacc")   # 2 banks
            i8 = [0]
            h1_tiles = {}
            h2_tiles = {}
            first_w4 = [True]

            def h1_write(dst, p, eng):
                if eng == "act":
                    nc.scalar.activation(dst, sb_qT2[:], Act.Relu,
                                         bias=kvb_col(p))
                else:
                    nc.vector.tensor_scalar(dst, sb_qT2[:], kvb_col(p), 0.0,
                                            Alu.add, Alu.max)

            def produce_h1(g):
                if g in fp8_groups:
                    h18 = h18pool.tile([128, 2, SH], f8, tag="h18")
                    for half in range(2):
                        h1_write(h18[:, half, :], 2 * g + half, eng8[i8[0]])
                        i8[0] += 1
                    h1_tiles[g] = h18
                else:
                    ts = []
                    for half in range(2):
                        p = 2 * g + half
                        h1t = h1pool.tile([128, SH], bf16, tag="h1")
                        h1_write(h1t[:], p, h1_eng[p])
                        ts.append(h1t)
                    h1_tiles[g] = ts

            def consume_group(g):
                ps_h = pmain.tile([128, SH], fp32, tag="hps")
                src = h1_tiles.pop(g)
                if g in fp8_groups:
                    for qc in range(2):
                        sl = slice(qc * 512, (qc + 1) * 512)
                        nc.tensor.matmul(ps_h[:, sl], sb_w28,
                                         src[:, :, sl], start=True, stop=True,
                                         perf_mode=DR)
                else:
                    for half in range(2):
                        rows = slice(half * H1, (half + 1) * H1)
                        for n in range(2):
                            sl = slice(n * 512, (n + 1) * 512)
                            nc.tensor.matmul(ps_h[rows, sl], sb_bdmw2,
                                             src[half][:, sl],
                                             start=True, stop=True)
                # h2: relu+bias -> fp8 into paired super-tile
                if g % 2 == 0:
                    h2_tiles[g] = h2pool.tile([128, 2, SH], f8, tag="h2d",
                                              name=f"h2d{g}")
                h2cur = h2_tiles[g - g % 2]
                dst = h2cur[:, g % 2, :]
                eng = h2_eng[g]
                if eng == "act":
                    nc.scalar.activation(dst, ps_h[:], Act.Relu, bias=sb_mb24)
                else:
                    nc.vector.tensor_scalar(dst, ps_h[:], sb_mb24, 0.0,
                                            Alu.add, Alu.max)
                # w4: fp8 DoubleRow over 8 tokens (2 groups)
                if g % 2 == 1:
                    h2cur = h2_tiles.pop(g - 1)
                    for qc in range(2):
                        sl = slice(qc * 512, (qc + 1) * 512)
                        nc.tensor.matmul(ps_out[:, sl], sb_w348,
                                         h2cur[:, :, sl],
                                         start=first_w4[0],
                                         stop=False, skip_group_check=True,
                                         perf_mode=DR)
                    first_w4[0] = False
                elif g == NG - 1:   # lone tail group: plain fp8 matmul
                    h2cur = h2_tiles.pop(g)
                    for qc in range(2):
                        sl = slice(qc * 512, (qc + 1) * 512)
                        nc.tensor.matmul(ps_out[:, sl], sb_w348[:, 0, :],
                                         h2cur[:, 0, sl],
                                         start=first_w4[0],
                                         stop=(qc == 1),
                                         skip_group_check=True)
                    first_w4[0] = False

            for gi in range(NG + LOOK):
                if gi < NG:
                    produce_h1(gi)
                if gi >= LOOK:
                    consume_group(gi - LOOK)

            # ---- stage F: scale + bias + store (host adds resid rows) ----
            sb_out = spool.tile([128, SH], fp32)
            nc.scalar.activation(sb_out[0:32, :], ps_out[0:32, :],
                                 Act.Identity, bias=sb_mb3x[0:32, :],
                                 scale=INV_K)
            nc.sync.dma_start(d_outT[:], sb_out[0:32, :])

    nc.compile()
    return nc


def _host_inputs(full, sw1, sb1, sw2, sb2, mw1, mb1, mw2, mb2, mw3, mb3):
    """Build the 8 per-core input maps (host-side sharding + layout prep)."""
    import ml_dtypes
    f32 = np.float32
    bf = ml_dtypes.bfloat16
    f8 = ml_dtypes.float8_e4m3

    full = np.asarray(full, dtype=f32)
    ones_row = np.ones((1, S), dtype=f32)

    c16 = np.zeros((128, C16_W), dtype=f32)
    c16[:, 0:KP] = np.arange(0, K, 2, dtype=f32)[None, :]
    c16[:, KP:K] = np.arange(1, K, 2, dtype=f32)[None, :]
    c16[0:DA, C16_SW1:C16_SW1 + H2] = np.concatenate(
        [np.asarray(sw1, f32), np.asarray(sb1, f32)[None, :]], axis=0)
    c16[0:H2, C16_SW2] = np.asarray(sw2, f32).reshape(H2)
    c16[0:DA, C16_WKV:C16_WKV + H1] = np.concatenate(
        [np.asarray(mw1[D:2 * D] + mw1[2 * D:], f32),
         np.asarray(mb1, f32)[None, :]], axis=0)
    c16[0:D, C16_WQ:C16_WQ + H1] = np.asarray(mw1[:D], f32)
    bd = np.zeros((128, H1), dtype=f32)
    bd[0:H1, 0:H2] = mw2
    bd[H1:128, H2:H1] = mw2
    c16[:, C16_BDMW2:C16_BDMW2 + H1] = bd
    c16 = c16.astype(bf)

    c32 = np.zeros((128, C32_W), dtype=f32)
    c32[:, C32_MB24] = np.tile(np.asarray(mb2, f32), 4)
    c32[0:D, C32_MB3X] = np.asarray(mb3, f32)
    c32[0, C32_ONE] = 1.0

    # fp8 weights: mw2 DoubleRow block + [q8(mw3); resid] DoubleRow block
    mw2_8 = np.asarray(mw2, f32).astype(f8)
    w28 = np.zeros((128, 2, 128), dtype=f8)
    w28[0:H1, 0, 0:32] = mw2_8
    w28[H1:128, 0, 32:64] = mw2_8
    w28[0:H1, 1, 64:96] = mw2_8
    w28[H1:128, 1, 96:128] = mw2_8
    mw3_8 = np.asarray(mw3, f32).astype(f8)
    mw3_r = (np.asarray(mw3, f32) - mw3_8.astype(f32)).astype(f8)
    w348 = np.zeros((128, 2, 128), dtype=f8)
    for t in range(2):
        w348[:, t, 0:16] = np.tile(mw3_8, (4, 1))
        w348[:, t, 16:32] = np.tile(mw3_r, (4, 1))
    c8 = np.concatenate([w28, w348], axis=2)

    shared = dict(c16=c16, c32=c32, c8=c8)
    in_maps = []
    for c in range(N_CORES):
        b, h = c // 2, c % 2
        fbT = np.concatenate(
            [np.ascontiguousarray(full[b].T), ones_row], axis=0)
        # token-major chunks with ones column: fb16[:, 17c:17c+17]
        fb = np.concatenate([full[b], np.ones((S, 1), np.float32)], axis=1)
        fb16 = np.ascontiguousarray(
            fb.reshape(NCH, 128, DA).transpose(1, 0, 2).reshape(128, NCH * DA))
        m = dict(shared)
        m["fbT16"] = fbT.astype(bf)
        m["fb16"] = fb16.astype(bf)
        m["fqT16"] = np.ascontiguousarray(
            full[b, h * SH:(h + 1) * SH, :].T).astype(bf)
        in_maps.append(m)
    return in_maps


def get_module():
    if "nc" not in _cache:
        _cache["nc"] = _build_module()
    return _cache["nc"]


def run_cores(in_maps):
    from concourse.bass_utils import run_bass_kernel_spmd
    nc = get_module()
    return run_bass_kernel_spmd(nc, in_maps, list(range(N_CORES))).results


def kernel(full, sw1, sb1, sw2, sb2, mw1, mb1, mw2, mb2, mw3, mb3):
    in_maps = _host_inputs(full, sw1, sb1, sw2, sb2, mw1, mb1, mw2, mb2,
                           mw3, mb3)
    results = run_cores(in_maps)
    out = np.empty((B, S, D), dtype=np.float32)
    for c in range(N_CORES):
        b, h = c // 2, c % 2
        oT = results[c]["outT"].astype(np.float32)
        out[b, h * SH:(h + 1) * SH, :] = (oT[0:D] + oT[D:2 * D]).T
    return out


# revision 56
# speedup vs baseline: 1.0030x; 1.0030x over previous
"""Trainium2 Bass kernel for nn_AttentionApproximator (sparse_attention).

Math (per batch b):
  scores = relu(full @ sw1 + sb1) @ sw2 + sb2            [S]
  top_idx = top_k(scores, k=204)                          (set only matters)
  sel     = full[top_idx]                                 [k, d]
  q_part  = full @ mw1[:d]                                [S, 64]
  kvb     = sel @ (mw1[d:2d] + mw1[2d:]) + mb1            [k, 64]
  h1      = relu(q_part[s] + kvb[j])                      [S, k, 64]
  h2      = relu(h1 @ mw2 + mb2)                          [S, k, 32]
  out     = mean_j(h2) @ mw3 + mb3                        [S, d]

Device strategy (8 cores, SPMD): core c handles batch b=c//2, query rows
h=c%2 (1024 of 2048).  Top-k via exact ranks (rank_i = #{j: s_j > s_i});
rank doubles as the compaction slot, gathered by one-hot matmul.

Perf structure vs the original version:
  - every stage matmul runs in bf16 (1 PE cycle/row instead of 4);
    scores stay self-consistent (fp32 PSUM accum, exact transposes).
  - h1 tiles are produced bf16 from a bf16 qT2 -> DVE 4x mode (327ns/op).
  - h2 is written as fp8 into paired super-tiles; the mw3 stage is a
    single fp8 DoubleRow matmul (half PE rate) whose stationary carries
    q8(mw3) in out-rows 0:16 and the quantization residual in 16:32;
    the host adds the residual rows after DMA.
  - the kv selection is fused: G = fb^T @ onehot gathers raw token
    features on the PE, then one wkv matmul yields the slot-pair kvb
    (no per-chunk kv matmuls or PSUM->SBUF copies).
  - scores broadcast via gpsimd partition_broadcast (the only legal
    Pool op here: the GPSIMD ISA has no tensor_scalar and cannot read
    PSUM); all other elementwise work is balanced across DVE and ACT.
  - h1 producers are emitted LOOK groups ahead so the PE stream never
    waits on its h1 inputs; engine splits are env-tunable knobs.
"""

import os
from contextlib import ExitStack

import numpy as np

B, S, D = 4, 2048, 16
DA = D + 1               # augmented with ones row
K = 204                  # top-k  (int(2048*0.1))
KP = K // 2              # 102 slot-pairs
H1 = 64
H2 = 32
SH = S // 2              # 1024 query rows per core
NCH = S // 128           # 16 token chunks
NG = KP // 2             # 51 groups of 4 tokens
N_CORES = 8
INV_K = float(np.float32(1.0) / np.float32(K))

# ---- tuning knobs ----
def _env(name, dflt):
    return int(os.environ.get(name, str(dflt)))

Z8Q = _env("KERNEL_Z8Q", 0)          # groups on the fp8-DoubleRow mw2 path
H18_ACT = _env("KERNEL_H18_ACT", 0)  # fp8 h1 halves made by ACT (rest DVE)
M_ACT = _env("KERNEL_M_ACT", 44)     # h2 jobs on ACT (of 51, rest DVE)
M_TAILD = _env("KERNEL_M_TAILD", 0)  # last groups' h2 forced to DVE
B_ACT = _env("KERNEL_B_ACT", 6)      # rank chunks on ACT (of 16, rest DVE)
H1_ACT = _env("KERNEL_H1_ACT", 0)    # bf16 h1 pairs on ACT (of 102, rest DVE)
LOOK = _env("KERNEL_LOOK", 3)        # h1 producer lookahead (groups)

# const-tile column layout (bf16 tensor)
C16_IOTA = 0             # [128, 204] one-hot iota (evens 0:102, odds 102:204)
C16_SW1 = 204            # [DA, 32]
C16_SW2 = 236            # [H2, 1]
C16_WKV = 237            # [DA, 64]
C16_WQ = 301             # [D, 64]
C16_BDMW2 = 365          # [128, 64]
C16_W = 429
# fp32 const tensor
C32_MB24 = 0             # [128, 1] mb2 tiled x4
C32_MB3X = 1             # [128, 1] rows 0:16 = mb3, else 0
C32_ONE = 2              # [1, 1] = 1.0 (transpose aux)
C32_WARM = 16            # [1, 512] zeros (PE warmup)
C32_W = 528

_cache = {}


def _spread(n, total):
    """n indices spread evenly over range(total)."""
    if n <= 0:
        return []
    return sorted({min(total - 1, int(round(i * total / n))) for i in range(n)})


def _build_module():
    import concourse.mybir as mybir
    import concourse.tile as tile
    from concourse import bacc

    fp32 = mybir.dt.float32
    bf16 = mybir.dt.bfloat16
    f8 = mybir.dt.float8e4
    Alu = mybir.AluOpType
    Act = mybir.ActivationFunctionType
    DR = mybir.MatmulPerfMode.DoubleRow

    nc = bacc.Bacc("TRN2", target_bir_lowering=False, debug=False,
                   num_devices=N_CORES)

    d_fbT16 = nc.dram_tensor("fbT16", [DA, S], bf16, kind="ExternalInput").ap()
    d_fb16 = nc.dram_tensor("fb16", [128, DA * NCH], bf16,
                            kind="ExternalInput").ap()
    d_fqT16 = nc.dram_tensor("fqT16", [D, SH], bf16, kind="ExternalInput").ap()
    d_c16 = nc.dram_tensor("c16", [128, C16_W], bf16, kind="ExternalInput").ap()
    d_c32 = nc.dram_tensor("c32", [128, C32_W], fp32, kind="ExternalInput").ap()
    d_c8 = nc.dram_tensor("c8", [128, 2, 256], f8, kind="ExternalInput").ap()
    d_outT = nc.dram_tensor("outT", [2 * D, SH], fp32,
                            kind="ExternalOutput").ap()

    fp8_groups = set(_spread(Z8Q, NG))
    # engine per fp8-h1 half (2 halves per fp8 group)
    n8 = 2 * len(fp8_groups)
    eng8 = ["dve"] * n8
    for i in _spread(min(H18_ACT, n8), n8):
        eng8[i] = "act"
    # engine per h2 job (group): ACT or DVE (Pool cannot read PSUM, and
    # the GPSIMD ISA has no tensor_scalar)
    h2_eng = ["dve"] * NG
    for g in _spread(M_ACT, NG):
        h2_eng[g] = "act"
    for g in range(NG - M_TAILD, NG):
        h2_eng[g] = "dve"     # DVE drains first; keep ACT off the tail
    # engine per rank chunk
    b_eng = ["dve"] * NCH
    for c in _spread(B_ACT, NCH):
        b_eng[c] = "act"
    # bf16 h1 pairs lifted to ACT to balance DVE
    h1_eng = ["dve"] * KP
    for p in _spread(H1_ACT, KP):
        h1_eng[p] = "act"

    with tile.TileContext(nc) as tc:
        with (
            ExitStack() as es,
            tc.tile_pool(name="const", bufs=1) as cpool,
            tc.tile_pool(name="sel", bufs=1) as spool,
            tc.tile_pool(name="scratch", bufs=3) as scpool,
            tc.tile_pool(name="h1p", bufs=2 * (LOOK + 2)) as h1pool,
            tc.tile_pool(name="h18p", bufs=LOOK + 2) as h18pool,
            tc.tile_pool(name="h2p", bufs=4) as h2pool,
        ):
            # ---- constants / inputs to SBUF (critical-path DMAs first) ----
            sb_c16 = cpool.tile([128, C16_W], bf16)
            nc.sync.dma_start(sb_c16[:], d_c16[:])
            sb_fbT16 = cpool.tile([DA, S], bf16)
            nc.sync.dma_start(sb_fbT16[:, 0:1024], d_fbT16[:, 0:1024])
            nc.sync.dma_start(sb_fbT16[:, 1024:2048], d_fbT16[:, 1024:2048])
            sb_fqT16 = cpool.tile([D, SH], bf16)
            nc.sync.dma_start(sb_fqT16[:], d_fqT16[:])
            sb_fb16 = cpool.tile([128, DA * NCH], bf16)
            nc.sync.dma_start(sb_fb16[:], d_fb16[:])
            sb_c32 = cpool.tile([128, C32_W], fp32)
            nc.sync.dma_start(sb_c32[:], d_c32[:])
            sb_c8 = cpool.tile([128, 2, 256], f8)
            nc.sync.dma_start(sb_c8[:], d_c8[:])

            sb_iota = sb_c16[:, C16_IOTA:C16_IOTA + K]
            sb_sw1 = sb_c16[0:DA, C16_SW1:C16_SW1 + H2]
            sb_sw2 = sb_c16[0:H2, C16_SW2:C16_SW2 + 1]
            sb_wkv = sb_c16[0:DA, C16_WKV:C16_WKV + H1]
            sb_wq = sb_c16[0:D, C16_WQ:C16_WQ + H1]
            sb_bdmw2 = sb_c16[:, C16_BDMW2:C16_BDMW2 + H1]
            sb_mb24 = sb_c32[:, C32_MB24:C32_MB24 + 1]
            sb_mb3x = sb_c32[:, C32_MB3X:C32_MB3X + 1]
            sb_one = sb_c32[0:1, C32_ONE:C32_ONE + 1]
            sb_w28 = sb_c8[:, :, 0:128]
            sb_w348 = sb_c8[:, :, 128:256]

            # ---- PE warmup: ramps the PE clock while input DMAs stream ----
            with tc.tile_pool(name="pswarm", bufs=1, space="PSUM") as pW:
                ps_w = pW.tile([1, 429], fp32)
                nc.tensor.matmul(ps_w[0:1, :], sb_c16[0:1, 0:1],
                                 sb_c16[0:1, 0:429],
                                 start=True, stop=True)

            # ---- stage A: score hidden layer + per-token scores ----
            # pbcast chunks are emitted right behind each score-row copy so
            # the Pool engine streams the broadcast while stage A finishes.
            sb_srs = []
            sb_bc = cpool.tile([128, S], fp32)
            with tc.tile_pool(name="psA", bufs=1, space="PSUM") as pA, \
                 tc.tile_pool(name="psA15", bufs=1, space="PSUM") as pA15:
                sb_Hs = []
                for n in range(4):
                    sl = slice(n * 512, (n + 1) * 512)
                    ps_Hn = pA.tile([H2, 512], fp32, tag=f"H{n}")
                    nc.tensor.matmul(ps_Hn[:], sb_sw1, sb_fbT16[:, sl],
                                     start=True, stop=True)
                    sb_Hn = spool.tile([H2, 512], bf16, tag=f"sbH{n}")
                    if n % 2 == 0:     # alternate engines: shorter serial chain
                        nc.scalar.activation(sb_Hn[:], ps_Hn[:], Act.Relu)
                    else:
                        nc.vector.tensor_scalar(sb_Hn[:], ps_Hn[:], 0.0, None,
                                                Alu.max)
                    sb_Hs.append(sb_Hn)
                # scores: fp32 PSUM accumulation of bf16 products; all
                # downstream layouts derive from these rows exactly.
                for n in range(4):
                    sl = slice(n * 512, (n + 1) * 512)
                    ps_srn = pA15.tile([1, 512], fp32, tag=f"sr{n}")
                    nc.tensor.matmul(ps_srn[:], sb_sw2, sb_Hs[n][:],
                                     start=True, stop=True)
                    sb_srn = spool.tile([1, 512], fp32, tag=f"sbsr{n}")
                    nc.scalar.copy(sb_srn[:], ps_srn[:])
                    sb_srs.append(sb_srn)
                    nc.gpsimd.partition_broadcast(sb_bc[:, sl], sb_srn[0:1, :])

            with tc.tile_pool(name="psA2", bufs=1, space="PSUM") as pA2:
                # scores token-major via PE transposes (exact), in 4-column
                # tiles so early rank chunks start before stage A drains
                sb_stoks, sb_nstoks = [], []
                for t in range(4):
                    ps_st = pA2.tile([128, 4], fp32, tag=f"pst{t}")
                    for i in range(4):
                        nc.tensor.transpose(
                            ps_st[:, i:i + 1],
                            sb_srs[t][0:1, i * 128:(i + 1) * 128],
                            sb_one)
                    sb_st = spool.tile([128, 4], fp32, tag=f"stok{t}")
                    nc.vector.tensor_copy(sb_st[:], ps_st[:])
                    sb_stoks.append(sb_st)
                    sb_nst = spool.tile([128, 4], fp32, tag=f"nstok{t}")
                    nc.vector.tensor_scalar(sb_nst[:], sb_st[:], -1.0, None,
                                            Alu.mult)
                    sb_nstoks.append(sb_nst)

                def stok_col(c):
                    return sb_stoks[c // 4][:, c % 4:c % 4 + 1]

                def nstok_col(c):
                    return sb_nstoks[c // 4][:, c % 4:c % 4 + 1]

            es_c = ExitStack()
            pCs = es_c.enter_context(tc.tile_pool(name="psCsel", bufs=1,
                                                  space="PSUM"))

            # ---- stage D matmuls (qT2 copy is emitted after stage B so it
            # doesn't block ACT's rank chunks) ----
            pD = es_c.enter_context(tc.tile_pool(name="psD", bufs=1,
                                                 space="PSUM"))
            ps_q = pD.tile([128, SH], fp32)
            for half in range(2):
                rows = slice(half * H1, (half + 1) * H1)
                for n in range(2):
                    sl = slice(n * 512, (n + 1) * 512)
                    nc.tensor.matmul(ps_q[rows, sl], sb_wq,
                                     sb_fqT16[:, sl], start=True,
                                     stop=True)

            # ---- stage B: exact ranks (token-major); the raw-feature
            # gather G = fb^T . onehot is emitted per-chunk so it streams
            # behind the ranks ----
            ps_G = pCs.tile([17, 256], fp32)   # 1 bank (uses [:, 0:K])

            def rank_chunk(c):
                rk = spool.tile([128, 1], fp32, tag=f"rank{c}")
                if b_eng[c] == "act":
                    scr = scpool.tile([128, S], fp32, tag="cmp_scr2")
                    rsgn = spool.tile([128, 1], fp32, tag=f"rsgn{c}")
                    nc.scalar.activation(scr[:], sb_bc[:], Act.Sign,
                                         bias=nstok_col(c),
                                         accum_out=rsgn[:])
                    nc.vector.tensor_scalar(rk[:], rsgn[:], 0.5,
                                            1023.5, Alu.mult, Alu.add)
                else:
                    scr = scpool.tile([128, S], fp32, tag="cmp_scr")
                    nc.vector.tensor_scalar(
                        scr[:], sb_bc[:], stok_col(c), 0.0,
                        Alu.is_gt, Alu.add, accum_out=rk[:])
                return rk

            for c in range(NCH):
                rk = rank_chunk(c)
                oh = scpool.tile([128, K], bf16, tag="oh")
                nc.vector.tensor_scalar(oh[:], sb_iota, rk[:], None,
                                        Alu.is_equal)
                nc.tensor.matmul(ps_G[:, 0:K],
                                 sb_fb16[:, c * DA:(c + 1) * DA], oh[:],
                                 start=(c == 0), stop=(c == NCH - 1))

            sb_qT2 = cpool.tile([128, SH], bf16)
            nc.scalar.copy(sb_qT2[:], ps_q[:])
            # selected raw features -> SBUF, then kvb for slot pairs via wkv:
            # evens land in rows 0:64, odds in rows 64:128 (strided moving AP)
            sb_G = spool.tile([17, K], bf16)
            nc.scalar.copy(sb_G[:], ps_G[:, 0:K])
            pK = es_c.enter_context(tc.tile_pool(name="psKvb", bufs=1,
                                                 space="PSUM"))
            ps_kvb = pK.tile([128, KP], fp32)
            nc.tensor.matmul(ps_kvb[0:H1, :], sb_wkv, sb_G[:, 0:KP],
                             start=True, stop=True)
            nc.tensor.matmul(ps_kvb[H1:128, :], sb_wkv, sb_G[:, KP:K],
                             start=True, stop=True)
            sb_kvb2a = spool.tile([128, 51], fp32)
            nc.scalar.copy(sb_kvb2a[:], ps_kvb[:, 0:51])
            sb_kvb2b = spool.tile([128, KP - 51], fp32)
            nc.scalar.copy(sb_kvb2b[:], ps_kvb[:, 51:KP])
            es_c.close()

            def kvb_col(p):
                return (sb_kvb2a[:, p:p + 1] if p < 51
                        else sb_kvb2b[:, p - 51:p - 50])

            pmain = es.enter_context(
                tc.tile_pool(name="main_psum", bufs=3, space="PSUM"))
            pout = es.enter_context(
                tc.tile_pool(name="out_psum", bufs=1, space="PSUM"))

            # ---- stage E: main pairwise loop over 51 groups of 4 tokens.
            # h1 producers are emitted LOOK groups ahead of their matmuls so
            # slow ACT/Pool h1 tiles never stall the PE stream. ----
            ps_out = pout.tile([128, SH], fp32, tag="outacc")   # 2 banks
            i8 = [0]
            h1_tiles = {}
            h2_tiles = {}
            first_w4 = [True]

            def h1_write(dst, p, eng):
                if eng == "act":
                    nc.scalar.activation(dst, sb_qT2[:], Act.Relu,
                                         bias=kvb_col(p))
                else:
                    nc.vector.tensor_scalar(dst, sb_qT2[:], kvb_col(p), 0.0,
                                            Alu.add, Alu.max)

            def produce_h1(g):
                if g in fp8_groups:
                    h18 = h18pool.tile([128, 2, SH], f8, tag="h18")
                    for half in range(2):
                        h1_write(h18[:, half, :], 2 * g + half, eng8[i8[0]])
                        i8[0] += 1
                    h1_tiles[g] = h18
                else:
                    ts = []
                    for half in range(2):
                        p = 2 * g + half
                        h1t = h1pool.tile([128, SH], bf16, tag="h1")
                        h1_write(h1t[:], p, h1_eng[p])
                        ts.append(h1t)
                    h1_tiles[g] = ts

            def consume_group(g):
                ps_h = pmain.tile([128, SH], fp32, tag="hps")
                src = h1_tiles.pop(g)
                if g in fp8_groups:
                    for qc in range(2):
                        sl = slice(qc * 512, (qc + 1) * 512)
                        nc.tensor.matmul(ps_h[:, sl], sb_w28,
                                         src[:, :, sl], start=True, stop=True,
                                         perf_mode=DR)
                else:
                    for half in range(2):
                        rows = slice(half * H1, (half + 1) * H1)
                        for n in range(2):
                            sl = slice(n * 512, (n + 1) * 512)
                            nc.tensor.matmul(ps_h[rows, sl], sb_bdmw2,
                                             src[half][:, sl],
                                             start=True, stop=True)
                # h2: relu+bias -> fp8 into paired super-tile
                if g % 2 == 0:
                    h2_tiles[g] = h2pool.tile([128, 2, SH], f8, tag="h2d",
                                              name=f"h2d{g}")
                h2cur = h2_tiles[g - g % 2]
                dst = h2cur[:, g % 2, :]
                eng = h2_eng[g]
                if eng == "act":
                    nc.scalar.activation(dst, ps_h[:], Act.Relu, bias=sb_mb24)
                else:
                    nc.vector.tensor_scalar(dst, ps_h[:], sb_mb24, 0.0,
                                            Alu.add, Alu.max)
                # w4: fp8 DoubleRow over 8 tokens (2 groups)
                if g % 2 == 1:
                    h2cur = h2_tiles.pop(g - 1)
                    for qc in range(2):
                        sl = slice(qc * 512, (qc + 1) * 512)
                        nc.tensor.matmul(ps_out[:, sl], sb_w348,
                                         h2cur[:, :, sl],
                                         start=first_w4[0],
                                         stop=False, skip_group_check=True,
                                         perf_mode=DR)
                    first_w4[0] = False
                elif g == NG - 1:   # lone tail group: plain fp8 matmul
                    h2cur = h2_tiles.pop(g)
                    for qc in range(2):
                        sl = slice(qc * 512, (qc + 1) * 512)
                        nc.tensor.matmul(ps_out[:, sl], sb_w348[:, 0, :],
                                         h2cur[:, 0, sl],
                                         start=first_w4[0],
                                         stop=(qc == 1),
                                         skip_group_check=True)
                    first_w4[0] = False

            for gi in range(NG + LOOK):
                if gi < NG:
                    produce_h1(gi)
                if gi >= LOOK:
                    consume_group(gi - LOOK)

            # ---- stage F: scale + bias + store (host adds resid rows) ----
            sb_out = spool.tile([128, SH], fp32)
            nc.scalar.activation(sb_out[0:32, :], ps_out[0:32, :],
                                 Act.Identity, bias=sb_mb3x[0:32, :],
                                 scale=INV_K)
            nc.sync.dma_start(d_outT[:], sb_out[0:32, :])

    nc.compile()
    return nc


def _host_inputs(full, sw1, sb1, sw2, sb2, mw1, mb1, mw2, mb2, mw3, mb3):
    """Build the 8 per-core input maps (host-side sharding + layout prep)."""
    import ml_dtypes
    f32 = np.float32
    bf = ml_dtypes.bfloat16
    f8 = ml_dtypes.float8_e4m3

    full = np.asarray(full, dtype=f32)
    ones_row = np.ones((1, S), dtype=f32)

    c16 = np.zeros((128, C16_W), dtype=f32)
    c16[:, 0:KP] = np.arange(0, K, 2, dtype=f32)[None, :]
    c16[:, KP:K] = np.arange(1, K, 2, dtype=f32)[None, :]
    c16[0:DA, C16_SW1:C16_SW1 + H2] = np.concatenate(
        [np.asarray(sw1, f32), np.asarray(sb1, f32)[None, :]], axis=0)
    c16[0:H2, C16_SW2] = np.asarray(sw2, f32).reshape(H2)
    c16[0:DA, C16_WKV:C16_WKV + H1] = np.concatenate(
        [np.asarray(mw1[D:2 * D] + mw1[2 * D:], f32),
         np.asarray(mb1, f32)[None, :]], axis=0)
    c16[0:D, C16_WQ:C16_WQ + H1] = np.asarray(mw1[:D], f32)
    bd = np.zeros((128, H1), dtype=f32)
    bd[0:H1, 0:H2] = mw2
    bd[H1:128, H2:H1] = mw2
    c16[:, C16_BDMW2:C16_BDMW2 + H1] = bd
    c16 = c16.astype(bf)

    c32 = np.zeros((128, C32_W), dtype=f32)
    c32[:, C32_MB24] = np.tile(np.asarray(mb2, f32), 4)
    c32[0:D, C32_MB3X] = np.asarray(mb3, f32)
    c32[0, C32_ONE] = 1.0

    # fp8 weights: mw2 DoubleRow block + [q8(mw3); resid] DoubleRow block
    mw2_8 = np.asarray(mw2, f32).astype(f8)
    w28 = np.zeros((128, 2, 128), dtype=f8)
    w28[0:H1, 0, 0:32] = mw2_8
    w28[H1:128, 0, 32:64] = mw2_8
    w28[0:H1, 1, 64:96] = mw2_8
    w28[H1:128, 1, 96:128] = mw2_8
    mw3_8 = np.asarray(mw3, f32).astype(f8)
    mw3_r = (np.asarray(mw3, f32) - mw3_8.astype(f32)).astype(f8)
    w348 = np.zeros((128, 2, 128), dtype=f8)
    for t in range(2):
        w348[:, t, 0:16] = np.tile(mw3_8, (4, 1))
        w348[:, t, 16:32] = np.tile(mw3_r, (4, 1))
    c8 = np.concatenate([w28, w348], axis=2)

    shared = dict(c16=c16, c32=c32, c8=c8)
    in_maps = []
    for c in range(N_CORES):
        b, h = c // 2, c % 2
        fbT = np.concatenate(
            [np.ascontiguousarray(full[b].T), ones_row], axis=0)
        # token-major chunks with ones column: fb16[:, 17c:17c+17]
        fb = np.concatenate([full[b], np.ones((S, 1), np.float32)], axis=1)
        fb16 = np.ascontiguousarray(
            fb.reshape(NCH, 128, DA).transpose(1, 0, 2).reshape(128, NCH * DA))
        m = dict(shared)
        m["fbT16"] = fbT.astype(bf)
        m["fb16"] = fb16.astype(bf)
        m["fqT16"] = np.ascontiguousarray(
            full[b, h * SH:(h + 1) * SH, :].T).astype(bf)
        in_maps.append(m)
    return in_maps


def get_module():
    if "nc" not in _cache:
        _cache["nc"] = _build_module()
    return _cache["nc"]


def run_cores(in_maps):
    from concourse.bass_utils import run_bass_kernel_spmd
    nc = get_module()
    return run_bass_kernel_spmd(nc, in_maps, list(range(N_CORES))).results


def kernel(full, sw1, sb1, sw2, sb2, mw1, mb1, mw2, mb2, mw3, mb3):
    in_maps = _host_inputs(full, sw1, sb1, sw2, sb2, mw1, mb1, mw2, mb2,
                           mw3, mb3)
    results = run_cores(in_maps)
    out = np.empty((B, S, D), dtype=np.float32)
    for c in range(N_CORES):
        b, h = c // 2, c % 2
        oT = results[c]["outT"].astype(np.float32)
        out[b, h * SH:(h + 1) * SH, :] = (oT[0:D] + oT[D:2 * D]).T
    return out


# revision 60
# speedup vs baseline: 1.0033x; 1.0002x over previous
"""Trainium2 Bass kernel for nn_AttentionApproximator (sparse_attention).

Math (per batch b):
  scores = relu(full @ sw1 + sb1) @ sw2 + sb2            [S]
  top_idx = top_k(scores, k=204)                          (set only matters)
  sel     = full[top_idx]                                 [k, d]
  q_part  = full @ mw1[:d]                                [S, 64]
  kvb     = sel @ (mw1[d:2d] + mw1[2d:]) + mb1            [k, 64]
  h1      = relu(q_part[s] + kvb[j])                      [S, k, 64]
  h2      = relu(h1 @ mw2 + mb2)                          [S, k, 32]
  out     = mean_j(h2) @ mw3 + mb3                        [S, d]

Device strategy (8 cores, SPMD): core c handles batch b=c//2, query rows
h=c%2 (1024 of 2048).  Top-k via exact ranks (rank_i = #{j: s_j > s_i});
rank doubles as the compaction slot, gathered by one-hot matmul.

Perf structure vs the original version:
  - every stage matmul runs in bf16 (1 PE cycle/row instead of 4);
    scores stay self-consistent (fp32 PSUM accum, exact transposes).
  - h1 tiles are produced bf16 from a bf16 qT2 -> DVE 4x mode (327ns/op).
  - h2 is written as fp8 into paired super-tiles; the mw3 stage is a
    single fp8 DoubleRow matmul (half PE rate) whose stationary carries
    q8(mw3) in out-rows 0:16 and the quantization residual in 16:32;
    the host adds the residual rows after DMA.
  - the kv selection is fused: G = fb^T @ onehot gathers raw token
    features on the PE, then one wkv matmul yields the slot-pair kvb
    (no per-chunk kv matmuls or PSUM->SBUF copies).
  - scores broadcast via gpsimd partition_broadcast (the only legal
    Pool op here: the GPSIMD ISA has no tensor_scalar and cannot read
    PSUM); all other elementwise work is balanced across DVE and ACT.
  - h1 producers are emitted LOOK groups ahead so the PE stream never
    waits on its h1 inputs; engine splits are env-tunable knobs.
"""

import os
from contextlib import ExitStack

import numpy as np

B, S, D = 4, 2048, 16
DA = D + 1               # augmented with ones row
K = 204                  # top-k  (int(2048*0.1))
KP = K // 2              # 102 slot-pairs
H1 = 64
H2 = 32
SH = S // 2              # 1024 query rows per core
NCH = S // 128           # 16 token chunks
NG = KP // 2             # 51 groups of 4 tokens
N_CORES = 8
INV_K = float(np.float32(1.0) / np.float32(K))

# ---- tuning knobs ----
def _env(name, dflt):
    return int(os.environ.get(name, str(dflt)))

Z8Q = _env("KERNEL_Z8Q", 0)          # groups on the fp8-DoubleRow mw2 path
H18_ACT = _env("KERNEL_H18_ACT", 0)  # fp8 h1 halves made by ACT (rest DVE)
M_ACT = _env("KERNEL_M_ACT", 44)     # h2 jobs on ACT (of 51, rest DVE)
M_TAILD = _env("KERNEL_M_TAILD", 0)  # last groups' h2 forced to DVE
B_ACT = _env("KERNEL_B_ACT", 6)      # rank chunks on ACT (of 16, rest DVE)
H1_ACT = _env("KERNEL_H1_ACT", 0)    # bf16 h1 pairs on ACT (of 102, rest DVE)
LOOK = _env("KERNEL_LOOK", 3)        # h1 producer lookahead (groups)

# const-tile column layout (bf16 tensor)
C16_IOTA = 0             # [128, 204] one-hot iota (evens 0:102, odds 102:204)
C16_SW1 = 204            # [DA, 32]
C16_SW2 = 236            # [H2, 1]
C16_WKV = 237            # [DA, 64]
C16_WQ = 301             # [D, 64]
C16_BDMW2 = 365          # [128, 64]
C16_W = 429
# fp32 const tensor
C32_MB24 = 0             # [128, 1] mb2 tiled x4
C32_MB3X = 1             # [128, 1] rows 0:16 = mb3, else 0
C32_ONE = 2              # [1, 1] = 1.0 (transpose aux)
C32_WARM = 16            # [1, 512] zeros (PE warmup)
C32_W = 528

_cache = {}


def _spread(n, total):
    """n indices spread evenly over range(total)."""
    if n <= 0:
        return []
    return sorted({min(total - 1, int(round(i * total / n))) for i in range(n)})


def _build_module():
    import concourse.mybir as mybir
    import concourse.tile as tile
    from concourse import bacc

    fp32 = mybir.dt.float32
    bf16 = mybir.dt.bfloat16
    f8 = mybir.dt.float8e4
    Alu = mybir.AluOpType
    Act = mybir.ActivationFunctionType
    DR = mybir.MatmulPerfMode.DoubleRow

    nc = bacc.Bacc("TRN2", target_bir_lowering=False, debug=False,
                   num_devices=N_CORES)

    d_fbT16 = nc.dram_tensor("fbT16", [DA, S], bf16, kind="ExternalInput").ap()
    d_fb16 = nc.dram_tensor("fb16", [128, DA * NCH], bf16,
                            kind="ExternalInput").ap()
    d_fqT16 = nc.dram_tensor("fqT16", [D, SH], bf16, kind="ExternalInput").ap()
    d_c16 = nc.dram_tensor("c16", [128, C16_W], bf16, kind="ExternalInput").ap()
    d_c32 = nc.dram_tensor("c32", [128, C32_W], fp32, kind="ExternalInput").ap()
    d_c8 = nc.dram_tensor("c8", [128, 2, 256], f8, kind="ExternalInput").ap()
    d_outT = nc.dram_tensor("outT", [2 * D, SH], fp32,
                            kind="ExternalOutput").ap()

    fp8_groups = set(_spread(Z8Q, NG))
    # engine per fp8-h1 half (2 halves per fp8 group)
    n8 = 2 * len(fp8_groups)
    eng8 = ["dve"] * n8
    for i in _spread(min(H18_ACT, n8), n8):
        eng8[i] = "act"
    # engine per h2 job (group): ACT or DVE (Pool cannot read PSUM, and
    # the GPSIMD ISA has no tensor_scalar)
    h2_eng = ["dve"] * NG
    for g in _spread(M_ACT, NG):
        h2_eng[g] = "act"
    for g in range(NG - M_TAILD, NG):
        h2_eng[g] = "dve"     # DVE drains first; keep ACT off the tail
    # engine per rank chunk
    b_eng = ["dve"] * NCH
    for c in _spread(B_ACT, NCH):
        b_eng[c] = "act"
    # bf16 h1 pairs lifted to ACT to balance DVE
    h1_eng = ["dve"] * KP
    for p in _spread(H1_ACT, KP):
        h1_eng[p] = "act"

    with tile.TileContext(nc) as tc:
        with (
            ExitStack() as es,
            tc.tile_pool(name="const", bufs=1) as cpool,
            tc.tile_pool(name="sel", bufs=1) as spool,
            tc.tile_pool(name="scratch", bufs=3) as scpool,
            tc.tile_pool(name="h1p", bufs=2 * (LOOK + 2)) as h1pool,
            tc.tile_pool(name="h18p", bufs=LOOK + 2) as h18pool,
            tc.tile_pool(name="h2p", bufs=4) as h2pool,
        ):
            # ---- constants / inputs to SBUF (critical-path DMAs first) ----
            sb_c16 = cpool.tile([128, C16_W], bf16)
            nc.sync.dma_start(sb_c16[:], d_c16[:])
            sb_fbT16 = cpool.tile([DA, S], bf16)
            nc.sync.dma_start(sb_fbT16[:, 0:1024], d_fbT16[:, 0:1024])
            nc.sync.dma_start(sb_fbT16[:, 1024:2048], d_fbT16[:, 1024:2048])
            sb_fqT16 = cpool.tile([D, SH], bf16)
            nc.sync.dma_start(sb_fqT16[:], d_fqT16[:])
            sb_fb16 = cpool.tile([128, DA * NCH], bf16)
            nc.sync.dma_start(sb_fb16[:], d_fb16[:])
            sb_c32 = cpool.tile([128, C32_W], fp32)
            nc.sync.dma_start(sb_c32[:], d_c32[:])
            sb_c8 = cpool.tile([128, 2, 256], f8)
            nc.sync.dma_start(sb_c8[:], d_c8[:])

            sb_iota = sb_c16[:, C16_IOTA:C16_IOTA + K]
            sb_sw1 = sb_c16[0:DA, C16_SW1:C16_SW1 + H2]
            sb_sw2 = sb_c16[0:H2, C16_SW2:C16_SW2 + 1]
            sb_wkv = sb_c16[0:DA, C16_WKV:C16_WKV + H1]
            sb_wq = sb_c16[0:D, C16_WQ:C16_WQ + H1]
            sb_bdmw2 = sb_c16[:, C16_BDMW2:C16_BDMW2 + H1]
            sb_mb24 = sb_c32[:, C32_MB24:C32_MB24 + 1]
            sb_mb3x = sb_c32[:, C32_MB3X:C32_MB3X + 1]
            sb_one = sb_c32[0:1, C32_ONE:C32_ONE + 1]
            sb_w28 = sb_c8[:, :, 0:128]
            sb_w348 = sb_c8[:, :, 128:256]

            # ---- PE warmup: ramps the PE clock while input DMAs stream ----
            with tc.tile_pool(name="pswarm", bufs=1, space="PSUM") as pW:
                ps_w = pW.tile([1, 429], fp32)
                nc.tensor.matmul(ps_w[0:1, :], sb_c16[0:1, 0:1],
                                 sb_c16[0:1, 0:429],
                                 start=True, stop=True)

            # ---- stage A: score hidden layer + per-token scores ----
            # pbcast chunks are emitted right behind each score-row copy so
            # the Pool engine streams the broadcast while stage A finishes.
            sb_srs = []
            sb_bc = cpool.tile([128, S], fp32)
            with tc.tile_pool(name="psA", bufs=1, space="PSUM") as pA, \
                 tc.tile_pool(name="psA15", bufs=1, space="PSUM") as pA15:
                sb_Hs = []
                for n in range(4):
                    sl = slice(n * 512, (n + 1) * 512)
                    ps_Hn = pA.tile([H2, 512], fp32, tag=f"H{n}")
                    nc.tensor.matmul(ps_Hn[:], sb_sw1, sb_fbT16[:, sl],
                                     start=True, stop=True)
                    sb_Hn = spool.tile([H2, 512], bf16, tag=f"sbH{n}")
                    if n % 2 == 0:     # alternate engines: shorter serial chain
                        nc.scalar.activation(sb_Hn[:], ps_Hn[:], Act.Relu)
                    else:
                        nc.vector.tensor_scalar(sb_Hn[:], ps_Hn[:], 0.0, None,
                                                Alu.max)
                    sb_Hs.append(sb_Hn)
                # scores: fp32 PSUM accumulation of bf16 products; all
                # downstream layouts derive from these rows exactly.
                for n in range(4):
                    sl = slice(n * 512, (n + 1) * 512)
                    ps_srn = pA15.tile([1, 512], fp32, tag=f"sr{n}")
                    nc.tensor.matmul(ps_srn[:], sb_sw2, sb_Hs[n][:],
                                     start=True, stop=True)
                    sb_srn = spool.tile([1, 512], fp32, tag=f"sbsr{n}")
                    nc.scalar.copy(sb_srn[:], ps_srn[:])
                    sb_srs.append(sb_srn)
                    nc.gpsimd.partition_broadcast(sb_bc[:, sl], sb_srn[0:1, :])

            with tc.tile_pool(name="psA2", bufs=1, space="PSUM") as pA2:
                # scores token-major via PE transposes (exact), in 4-column
                # tiles so early rank chunks start before stage A drains
                sb_stoks, sb_nstoks = [], []
                for t in range(4):
                    ps_st = pA2.tile([128, 4], fp32, tag=f"pst{t}")
                    for i in range(4):
                        nc.tensor.transpose(
                            ps_st[:, i:i + 1],
                            sb_srs[t][0:1, i * 128:(i + 1) * 128],
                            sb_one)
                    sb_st = spool.tile([128, 4], fp32, tag=f"stok{t}")
                    nc.vector.tensor_copy(sb_st[:], ps_st[:])
                    sb_stoks.append(sb_st)
                    sb_nst = spool.tile([128, 4], fp32, tag=f"nstok{t}")
                    nc.vector.tensor_scalar(sb_nst[:], sb_st[:], -1.0, None,
                                            Alu.mult)
                    sb_nstoks.append(sb_nst)

                def stok_col(c):
                    return sb_stoks[c // 4][:, c % 4:c % 4 + 1]

                def nstok_col(c):
                    return sb_nstoks[c // 4][:, c % 4:c % 4 + 1]

            es_c = ExitStack()
            pCs = es_c.enter_context(tc.tile_pool(name="psCsel", bufs=1,
                                                  space="PSUM"))

            # ---- stage D matmuls (qT2 copy is emitted after stage B so it
            # doesn't block ACT's rank chunks) ----
            pD = es_c.enter_context(tc.tile_pool(name="psD", bufs=1,
                                                 space="PSUM"))
            ps_q = pD.tile([128, SH], fp32)
            for half in range(2):
                rows = slice(half * H1, (half + 1) * H1)
                for n in range(2):
                    sl = slice(n * 512, (n + 1) * 512)
                    nc.tensor.matmul(ps_q[rows, sl], sb_wq,
                                     sb_fqT16[:, sl], start=True,
                                     stop=True)

            # ---- stage B: exact ranks (token-major); the raw-feature
            # gather G = fb^T . onehot is emitted per-chunk so it streams
            # behind the ranks ----
            ps_G = pCs.tile([17, 256], fp32)   # 1 bank (uses [:, 0:K])

            def rank_chunk(c):
                rk = spool.tile([128, 1], fp32, tag=f"rank{c}")
                if b_eng[c] == "act":
                    scr = scpool.tile([128, S], fp32, tag="cmp_scr2")
                    rsgn = spool.tile([128, 1], fp32, tag=f"rsgn{c}")
                    nc.scalar.activation(scr[:], sb_bc[:], Act.Sign,
                                         bias=nstok_col(c),
                                         accum_out=rsgn[:])
                    nc.vector.tensor_scalar(rk[:], rsgn[:], 0.5,
                                            1023.5, Alu.mult, Alu.add)
                else:
                    scr = scpool.tile([128, S], fp32, tag="cmp_scr")
                    nc.vector.tensor_scalar(
                        scr[:], sb_bc[:], stok_col(c), 0.0,
                        Alu.is_gt, Alu.add, accum_out=rk[:])
                return rk

            for c in range(NCH):
                rk = rank_chunk(c)
                oh = scpool.tile([128, K], bf16, tag="oh")
                nc.vector.tensor_scalar(oh[:], sb_iota, rk[:], None,
                                        Alu.is_equal)
                nc.tensor.matmul(ps_G[:, 0:K],
                                 sb_fb16[:, c * DA:(c + 1) * DA], oh[:],
                                 start=(c == 0), stop=(c == NCH - 1))

            sb_qT2 = cpool.tile([128, SH], bf16)
            nc.scalar.copy(sb_qT2[:], ps_q[:])
            # selected raw features -> SBUF, then kvb for slot pairs via wkv:
            # evens land in rows 0:64, odds in rows 64:128 (strided moving AP)
            sb_G = spool.tile([17, K], bf16)
            nc.vector.tensor_copy(sb_G[:], ps_G[:, 0:K])
            pK = es_c.enter_context(tc.tile_pool(name="psKvb", bufs=1,
                                                 space="PSUM"))
            ps_kvb = pK.tile([128, KP], fp32)
            nc.tensor.matmul(ps_kvb[0:H1, :], sb_wkv, sb_G[:, 0:KP],
                             start=True, stop=True)
            nc.tensor.matmul(ps_kvb[H1:128, :], sb_wkv, sb_G[:, KP:K],
                             start=True, stop=True)
            sb_kvb2a = spool.tile([128, 51], fp32)
            nc.vector.tensor_copy(sb_kvb2a[:], ps_kvb[:, 0:51])
            sb_kvb2b = spool.tile([128, KP - 51], fp32)
            nc.vector.tensor_copy(sb_kvb2b[:], ps_kvb[:, 51:KP])
            es_c.close()

            def kvb_col(p):
                return (sb_kvb2a[:, p:p + 1] if p < 51
                        else sb_kvb2b[:, p - 51:p - 50])

            pmain = es.enter_context(
                tc.tile_pool(name="main_psum", bufs=3, space="PSUM"))
            pout = es.enter_context(
                tc.tile_pool(name="out_psum", bufs=1, space="PSUM"))

            # ---- stage E: main pairwise loop over 51 groups of 4 tokens.
            # h1 producers are emitted LOOK groups ahead of their matmuls so
            # slow ACT/Pool h1 tiles never stall the PE stream. ----
            ps_out = pout.tile([128, SH], fp32, tag="outacc")   # 2 banks
            i8 = [0]
            h1_tiles = {}
            h2_tiles = {}
            first_w4 = [True]

            def h1_write(dst, p, eng):
                if eng == "act":
                    nc.scalar.activation(dst, sb_qT2[:], Act.Relu,
                                         bias=kvb_col(p))
                else:
                    nc.vector.tensor_scalar(dst, sb_qT2[:], kvb_col(p), 0.0,
                                            Alu.add, Alu.max)

            # ACT-assigned bf16 h1 pairs are emitted UPFRONT: they slot into
            # ACT's idle window between its rank chunks and the h2 stream,
            # instead of convoying the PE mid-loop.
            pre_tiles = {}
            for p in range(KP):
                if h1_eng[p] == "act" and p // 2 not in fp8_groups:
                    h1p = h1pool.tile([128, SH], bf16, tag="h1pre",
                                      name=f"h1pre{p}")
                    nc.scalar.activation(h1p[:], sb_qT2[:], Act.Relu,
                                         bias=kvb_col(p))
                    pre_tiles[p] = h1p

            def produce_h1(g):
                if g in fp8_groups:
                    h18 = h18pool.tile([128, 2, SH], f8, tag="h18")
                    for half in range(2):
                        h1_write(h18[:, half, :], 2 * g + half, eng8[i8[0]])
                        i8[0] += 1
                    h1_tiles[g] = h18
                else:
                    ts = []
                    for half in range(2):
                        p = 2 * g + half
                        if p in pre_tiles:
                            ts.append(pre_tiles.pop(p))
                            continue
                        h1t = h1pool.tile([128, SH], bf16, tag="h1")
                        h1_write(h1t[:], p, h1_eng[p])
                        ts.append(h1t)
                    h1_tiles[g] = ts

            def consume_group(g):
                ps_h = pmain.tile([128, SH], fp32, tag="hps")
                src = h1_tiles.pop(g)
                if g in fp8_groups:
                    for qc in range(2):
                        sl = slice(qc * 512, (qc + 1) * 512)
                        nc.tensor.matmul(ps_h[:, sl], sb_w28,
                                         src[:, :, sl], start=True, stop=True,
                                         perf_mode=DR)
                else:
                    for half in range(2):
                        rows = slice(half * H1, (half + 1) * H1)
                        for n in range(2):
                            sl = slice(n * 512, (n + 1) * 512)
                            nc.tensor.matmul(ps_h[rows, sl], sb_bdmw2,
                                             src[half][:, sl],
                                             start=True, stop=True)
                # h2: relu+bias -> fp8 into paired super-tile
                if g % 2 == 0:
                    h2_tiles[g] = h2pool.tile([128, 2, SH], f8, tag="h2d",
                                              name=f"h2d{g}")
                h2cur = h2_tiles[g - g % 2]
                dst = h2cur[:, g % 2, :]
                eng = h2_eng[g]
                if eng == "act":
                    nc.scalar.activation(dst, ps_h[:], Act.Relu, bias=sb_mb24)
                else:
                    nc.vector.tensor_scalar(dst, ps_h[:], sb_mb24, 0.0,
                                            Alu.add, Alu.max)
                # w4: fp8 DoubleRow over 8 tokens (2 groups)
                if g % 2 == 1:
                    h2cur = h2_tiles.pop(g - 1)
                    for qc in range(2):
                        sl = slice(qc * 512, (qc + 1) * 512)
                        nc.tensor.matmul(ps_out[:, sl], sb_w348,
                                         h2cur[:, :, sl],
                                         start=first_w4[0],
                                         stop=False, skip_group_check=True,
                                         perf_mode=DR)
                    first_w4[0] = False
                elif g == NG - 1:   # lone tail group: plain fp8 matmul
                    h2cur = h2_tiles.pop(g)
                    for qc in range(2):
                        sl = slice(qc * 512, (qc + 1) * 512)
                        nc.tensor.matmul(ps_out[:, sl], sb_w348[:, 0, :],
                                         h2cur[:, 0, sl],
                                         start=first_w4[0],
                                         stop=(qc == 1),
                                         skip_group_check=True)
                    first_w4[0] = False

            for gi in range(NG + LOOK):
                if gi < NG:
                    produce_h1(gi)
                if gi >= LOOK:
                    consume_group(gi - LOOK)

            # ---- stage F: scale + bias + store (host adds resid rows) ----
            sb_out = spool.tile([128, SH], fp32)
            nc.scalar.activation(sb_out[0:32, :], ps_out[0:32, :],
                                 Act.Identity, bias=sb_mb3x[0:32, :],
                                 scale=INV_K)
            nc.sync.dma_start(d_outT[:], sb_out[0:32, :])

    nc.compile()
    return nc


def _host_inputs(full, sw1, sb1, sw2, sb2, mw1, mb1, mw2, mb2, mw3, mb3):
    """Build the 8 per-core input maps (host-side sharding + layout prep)."""
    import ml_dtypes
    f32 = np.float32
    bf = ml_dtypes.bfloat16
    f8 = ml_dtypes.float8_e4m3

    full = np.asarray(full, dtype=f32)
    ones_row = np.ones((1, S), dtype=f32)

    c16 = np.zeros((128, C16_W), dtype=f32)
    c16[:, 0:KP] = np.arange(0, K, 2, dtype=f32)[None, :]
    c16[:, KP:K] = np.arange(1, K, 2, dtype=f32)[None, :]
    c16[0:DA, C16_SW1:C16_SW1 + H2] = np.concatenate(
        [np.asarray(sw1, f32), np.asarray(sb1, f32)[None, :]], axis=0)
    c16[0:H2, C16_SW2] = np.asarray(sw2, f32).reshape(H2)
    c16[0:DA, C16_WKV:C16_WKV + H1] = np.concatenate(
        [np.asarray(mw1[D:2 * D] + mw1[2 * D:], f32),
         np.asarray(mb1, f32)[None, :]], axis=0)
    c16[0:D, C16_WQ:C16_WQ + H1] = np.asarray(mw1[:D], f32)
    bd = np.zeros((128, H1), dtype=f32)
    bd[0:H1, 0:H2] = mw2
    bd[H1:128, H2:H1] = mw2
    c16[:, C16_BDMW2:C16_BDMW2 + H1] = bd
    c16 = c16.astype(bf)

    c32 = np.zeros((128, C32_W), dtype=f32)
    c32[:, C32_MB24] = np.tile(np.asarray(mb2, f32), 4)
    c32[0:D, C32_MB3X] = np.asarray(mb3, f32)
    c32[0, C32_ONE] = 1.0

    # fp8 weights: mw2 DoubleRow block + [q8(mw3); resid] DoubleRow block
    mw2_8 = np.asarray(mw2, f32).astype(f8)
    w28 = np.zeros((128, 2, 128), dtype=f8)
    w28[0:H1, 0, 0:32] = mw2_8
    w28[H1:128, 0, 32:64] = mw2_8
    w28[0:H1, 1, 64:96] = mw2_8
    w28[H1:128, 1, 96:128] = mw2_8
    mw3_8 = np.asarray(mw3, f32).astype(f8)
    mw3_r = (np.asarray(mw3, f32) - mw3_8.astype(f32)).astype(f8)
    w348 = np.zeros((128, 2, 128), dtype=f8)
    for t in range(2):
        w348[:, t, 0:16] = np.tile(mw3_8, (4, 1))
        w348[:, t, 16:32] = np.tile(mw3_r, (4, 1))
    c8 = np.concatenate([w28, w348], axis=2)

    shared = dict(c16=c16, c32=c32, c8=c8)
    in_maps = []
    for c in range(N_CORES):
        b, h = c // 2, c % 2
        fbT = np.concatenate(
            [np.ascontiguousarray(full[b].T), ones_row], axis=0)
        # token-major chunks with ones column: fb16[:, 17c:17c+17]
        fb = np.concatenate([full[b], np.ones((S, 1), np.float32)], axis=1)
        fb16 = np.ascontiguousarray(
            fb.reshape(NCH, 128, DA).transpose(1, 0, 2).reshape(128, NCH * DA))
        m = dict(shared)
        m["fbT16"] = fbT.astype(bf)
        m["fb16"] = fb16.astype(bf)
        m["fqT16"] = np.ascontiguousarray(
            full[b, h * SH:(h + 1) * SH, :].T).astype(bf)
        in_maps.append(m)
    return in_maps


def get_module():
    if "nc" not in _cache:
        _cache["nc"] = _build_module()
    return _cache["nc"]


def run_cores(in_maps):
    from concourse.bass_utils import run_bass_kernel_spmd
    nc = get_module()
    return run_bass_kernel_spmd(nc, in_maps, list(range(N_CORES))).results


def kernel(full, sw1, sb1, sw2, sb2, mw1, mb1, mw2, mb2, mw3, mb3):
    in_maps = _host_inputs(full, sw1, sb1, sw2, sb2, mw1, mb1, mw2, mb2,
                           mw3, mb3)
    results = run_cores(in_maps)
    out = np.empty((B, S, D), dtype=np.float32)
    for c in range(N_CORES):
        b, h = c // 2, c % 2
        oT = results[c]["outT"].astype(np.float32)
        out[b, h * SH:(h + 1) * SH, :] = (oT[0:D] + oT[D:2 * D]).T
    return out


# revision 63
# speedup vs baseline: 1.0151x; 1.0118x over previous
"""Trainium2 Bass kernel for nn_AttentionApproximator (sparse_attention).

Math (per batch b):
  scores = relu(full @ sw1 + sb1) @ sw2 + sb2            [S]
  top_idx = top_k(scores, k=204)                          (set only matters)
  sel     = full[top_idx]                                 [k, d]
  q_part  = full @ mw1[:d]                                [S, 64]
  kvb     = sel @ (mw1[d:2d] + mw1[2d:]) + mb1            [k, 64]
  h1      = relu(q_part[s] + kvb[j])                      [S, k, 64]
  h2      = relu(h1 @ mw2 + mb2)                          [S, k, 32]
  out     = mean_j(h2) @ mw3 + mb3                        [S, d]

Device strategy (8 cores, SPMD): core c handles batch b=c//2, query rows
h=c%2 (1024 of 2048).  Top-k via exact ranks (rank_i = #{j: s_j > s_i});
rank doubles as the compaction slot, gathered by one-hot matmul.

Perf structure vs the original version:
  - every stage matmul runs in bf16 (1 PE cycle/row instead of 4);
    scores stay self-consistent (fp32 PSUM accum, exact transposes).
  - h1 tiles are produced bf16 from a bf16 qT2 -> DVE 4x mode (327ns/op).
  - h2 is written as fp8 into paired super-tiles; the mw3 stage is a
    single fp8 DoubleRow matmul (half PE rate) whose stationary carries
    q8(mw3) in out-rows 0:16 and the quantization residual in 16:32;
    the host adds the residual rows after DMA.
  - the kv selection is fused: G = fb^T @ onehot gathers raw token
    features on the PE, then one wkv matmul yields the slot-pair kvb
    (no per-chunk kv matmuls or PSUM->SBUF copies).
  - scores broadcast via gpsimd partition_broadcast (the only legal
    Pool op here: the GPSIMD ISA has no tensor_scalar and cannot read
    PSUM); all other elementwise work is balanced across DVE and ACT.
  - h1 producers are emitted LOOK groups ahead so the PE stream never
    waits on its h1 inputs; engine splits are env-tunable knobs.
"""

import os
from contextlib import ExitStack

import numpy as np

B, S, D = 4, 2048, 16
DA = D + 1               # augmented with ones row
K = 204                  # top-k  (int(2048*0.1))
KP = K // 2              # 102 slot-pairs
H1 = 64
H2 = 32
SH = S // 2              # 1024 query rows per core
NCH = S // 128           # 16 token chunks
NG = KP // 2             # 51 groups of 4 tokens
N_CORES = 8
INV_K = float(np.float32(1.0) / np.float32(K))

# ---- tuning knobs ----
def _env(name, dflt):
    return int(os.environ.get(name, str(dflt)))

Z8Q = _env("KERNEL_Z8Q", 0)          # groups on the fp8-DoubleRow mw2 path
H18_ACT = _env("KERNEL_H18_ACT", 0)  # fp8 h1 halves made by ACT (rest DVE)
M_ACT = _env("KERNEL_M_ACT", 45)     # h2 jobs on ACT (of 51, rest DVE)
M_TAILD = _env("KERNEL_M_TAILD", 0)  # last groups' h2 forced to DVE
B_ACT = _env("KERNEL_B_ACT", 6)      # rank chunks on ACT (of 16, rest DVE)
H1_ACT = _env("KERNEL_H1_ACT", 0)    # bf16 h1 pairs on ACT (of 102, rest DVE)
LOOK = _env("KERNEL_LOOK", 3)        # h1 producer lookahead (groups)

# const-tile column layout (bf16 tensor)
C16_IOTA = 0             # [128, 204] one-hot iota (evens 0:102, odds 102:204)
C16_SW1 = 204            # [DA, 32]
C16_SW2 = 236            # [H2, 1]
C16_WKV = 237            # [DA, 64]
C16_WQ = 301             # [D, 64]
C16_BDMW2 = 365          # [128, 64]
C16_W = 429
# fp32 const tensor
C32_MB24 = 0             # [128, 1] mb2 tiled x4
C32_MB3X = 1             # [128, 1] rows 0:16 = mb3, else 0
C32_ONE = 2              # [1, 1] = 1.0 (transpose aux)
C32_WARM = 16            # [1, 512] zeros (PE warmup)
C32_W = 528

_cache = {}


def _spread(n, total):
    """n indices spread evenly over range(total)."""
    if n <= 0:
        return []
    return sorted({min(total - 1, int(round(i * total / n))) for i in range(n)})


def _build_module():
    import concourse.mybir as mybir
    import concourse.tile as tile
    from concourse import bacc

    fp32 = mybir.dt.float32
    bf16 = mybir.dt.bfloat16
    f8 = mybir.dt.float8e4
    Alu = mybir.AluOpType
    Act = mybir.ActivationFunctionType
    DR = mybir.MatmulPerfMode.DoubleRow

    nc = bacc.Bacc("TRN2", target_bir_lowering=False, debug=False,
                   num_devices=N_CORES)

    d_fbT16 = nc.dram_tensor("fbT16", [DA, S], bf16, kind="ExternalInput").ap()
    d_fb16 = nc.dram_tensor("fb16", [128, DA * NCH], bf16,
                            kind="ExternalInput").ap()
    d_fqT16 = nc.dram_tensor("fqT16", [D, SH], bf16, kind="ExternalInput").ap()
    d_c16 = nc.dram_tensor("c16", [128, C16_W], bf16, kind="ExternalInput").ap()
    d_c32 = nc.dram_tensor("c32", [128, C32_W], fp32, kind="ExternalInput").ap()
    d_c8 = nc.dram_tensor("c8", [128, 2, 256], f8, kind="ExternalInput").ap()
    d_outT = nc.dram_tensor("outT", [2 * D, SH], fp32,
                            kind="ExternalOutput").ap()

    fp8_groups = set(_spread(Z8Q, NG))
    # engine per fp8-h1 half (2 halves per fp8 group)
    n8 = 2 * len(fp8_groups)
    eng8 = ["dve"] * n8
    for i in _spread(min(H18_ACT, n8), n8):
        eng8[i] = "act"
    # engine per h2 job (group): ACT or DVE (Pool cannot read PSUM, and
    # the GPSIMD ISA has no tensor_scalar)
    h2_eng = ["dve"] * NG
    for g in _spread(M_ACT, NG):
        h2_eng[g] = "act"
    for g in range(NG - M_TAILD, NG):
        h2_eng[g] = "dve"     # DVE drains first; keep ACT off the tail
    # engine per rank chunk
    b_eng = ["dve"] * NCH
    for c in _spread(B_ACT, NCH):
        b_eng[c] = "act"
    # bf16 h1 pairs lifted to ACT to balance DVE
    h1_eng = ["dve"] * KP
    for p in _spread(H1_ACT, KP):
        h1_eng[p] = "act"

    with tile.TileContext(nc) as tc:
        with (
            ExitStack() as es,
            tc.tile_pool(name="const", bufs=1) as cpool,
            tc.tile_pool(name="sel", bufs=1) as spool,
            tc.tile_pool(name="scratch", bufs=3) as scpool,
            tc.tile_pool(name="h1p", bufs=2 * (LOOK + 2)) as h1pool,
            tc.tile_pool(name="h18p", bufs=LOOK + 2) as h18pool,
            tc.tile_pool(name="h2p", bufs=4) as h2pool,
        ):
            # ---- constants / inputs to SBUF (critical-path DMAs first) ----
            sb_c16 = cpool.tile([128, C16_W], bf16)
            nc.sync.dma_start(sb_c16[:], d_c16[:])
            sb_fbT16 = cpool.tile([DA, S], bf16)
            nc.sync.dma_start(sb_fbT16[:, 0:1024], d_fbT16[:, 0:1024])
            nc.sync.dma_start(sb_fbT16[:, 1024:2048], d_fbT16[:, 1024:2048])
            sb_fqT16 = cpool.tile([D, SH], bf16)
            nc.sync.dma_start(sb_fqT16[:], d_fqT16[:])
            sb_fb16 = cpool.tile([128, DA * NCH], bf16)
            nc.sync.dma_start(sb_fb16[:], d_fb16[:])
            sb_c32 = cpool.tile([128, C32_W], fp32)
            nc.sync.dma_start(sb_c32[:], d_c32[:])
            sb_c8 = cpool.tile([128, 2, 256], f8)
            nc.sync.dma_start(sb_c8[:], d_c8[:])

            sb_iota = sb_c16[:, C16_IOTA:C16_IOTA + K]
            sb_sw1 = sb_c16[0:DA, C16_SW1:C16_SW1 + H2]
            sb_sw2 = sb_c16[0:H2, C16_SW2:C16_SW2 + 1]
            sb_wkv = sb_c16[0:DA, C16_WKV:C16_WKV + H1]
            sb_wq = sb_c16[0:D, C16_WQ:C16_WQ + H1]
            sb_bdmw2 = sb_c16[:, C16_BDMW2:C16_BDMW2 + H1]
            sb_mb24 = sb_c32[:, C32_MB24:C32_MB24 + 1]
            sb_mb3x = sb_c32[:, C32_MB3X:C32_MB3X + 1]
            sb_one = sb_c32[0:1, C32_ONE:C32_ONE + 1]
            sb_w28 = sb_c8[:, :, 0:128]
            sb_w348 = sb_c8[:, :, 128:256]

            # ---- PE warmup: a memset tile lets the ramp start at t~0,
            # before any input DMA lands, so stage A runs at full clock ----
            with tc.tile_pool(name="pswarm", bufs=1, space="PSUM") as pW:
                sb_wz = spool.tile([1, 512], bf16)
                nc.vector.memzero(sb_wz[:])
                ps_w = pW.tile([1, 512], fp32)
                nc.tensor.matmul(ps_w[0:1, :], sb_wz[0:1, 0:1],
                                 sb_wz[0:1, :],
                                 start=True, stop=True)

            # ---- stage A: score hidden layer + per-token scores ----
            # pbcast chunks are emitted right behind each score-row copy so
            # the Pool engine streams the broadcast while stage A finishes.
            sb_srs = []
            sb_bc = cpool.tile([128, S], fp32)
            with tc.tile_pool(name="psA", bufs=1, space="PSUM") as pA, \
                 tc.tile_pool(name="psA15", bufs=1, space="PSUM") as pA15:
                sb_Hs = []
                for n in range(4):
                    sl = slice(n * 512, (n + 1) * 512)
                    ps_Hn = pA.tile([H2, 512], fp32, tag=f"H{n}")
                    nc.tensor.matmul(ps_Hn[:], sb_sw1, sb_fbT16[:, sl],
                                     start=True, stop=True)
                    sb_Hn = spool.tile([H2, 512], bf16, tag=f"sbH{n}")
                    if n % 2 == 0:     # alternate engines: shorter serial chain
                        nc.scalar.activation(sb_Hn[:], ps_Hn[:], Act.Relu)
                    else:
                        nc.vector.tensor_scalar(sb_Hn[:], ps_Hn[:], 0.0, None,
                                                Alu.max)
                    sb_Hs.append(sb_Hn)
                # scores: fp32 PSUM accumulation of bf16 products; all
                # downstream layouts derive from these rows exactly.
                for n in range(4):
                    sl = slice(n * 512, (n + 1) * 512)
                    ps_srn = pA15.tile([1, 512], fp32, tag=f"sr{n}")
                    nc.tensor.matmul(ps_srn[:], sb_sw2, sb_Hs[n][:],
                                     start=True, stop=True)
                    sb_srn = spool.tile([1, 512], fp32, tag=f"sbsr{n}")
                    if n % 2 == 0:     # alternate engines: shorter serial chain
                        nc.scalar.copy(sb_srn[:], ps_srn[:])
                    else:
                        nc.vector.tensor_copy(sb_srn[:], ps_srn[:])
                    sb_srs.append(sb_srn)
                    nc.gpsimd.partition_broadcast(sb_bc[:, sl], sb_srn[0:1, :])

            with tc.tile_pool(name="psA2", bufs=1, space="PSUM") as pA2:
                # scores token-major via PE transposes (exact), in 4-column
                # tiles so early rank chunks start before stage A drains
                sb_stoks, sb_nstoks = [], []
                for t in range(4):
                    ps_st = pA2.tile([128, 4], fp32, tag=f"pst{t}")
                    for i in range(4):
                        nc.tensor.transpose(
                            ps_st[:, i:i + 1],
                            sb_srs[t][0:1, i * 128:(i + 1) * 128],
                            sb_one)
                    sb_st = spool.tile([128, 4], fp32, tag=f"stok{t}")
                    nc.vector.tensor_copy(sb_st[:], ps_st[:])
                    sb_stoks.append(sb_st)
                    sb_nst = spool.tile([128, 4], fp32, tag=f"nstok{t}")
                    nc.vector.tensor_scalar(sb_nst[:], sb_st[:], -1.0, None,
                                            Alu.mult)
                    sb_nstoks.append(sb_nst)

                def stok_col(c):
                    return sb_stoks[c // 4][:, c % 4:c % 4 + 1]

                def nstok_col(c):
                    return sb_nstoks[c // 4][:, c % 4:c % 4 + 1]

            es_c = ExitStack()
            pCs = es_c.enter_context(tc.tile_pool(name="psCsel", bufs=1,
                                                  space="PSUM"))

            # ---- stage D matmuls (qT2 copy is emitted after stage B so it
            # doesn't block ACT's rank chunks) ----
            pD = es_c.enter_context(tc.tile_pool(name="psD", bufs=1,
                                                 space="PSUM"))
            ps_q = pD.tile([128, SH], fp32)
            for half in range(2):
                rows = slice(half * H1, (half + 1) * H1)
                for n in range(2):
                    sl = slice(n * 512, (n + 1) * 512)
                    nc.tensor.matmul(ps_q[rows, sl], sb_wq,
                                     sb_fqT16[:, sl], start=True,
                                     stop=True)

            # ---- stage B: exact ranks (token-major); the raw-feature
            # gather G = fb^T . onehot is emitted per-chunk so it streams
            # behind the ranks ----
            ps_G = pCs.tile([17, 256], fp32)   # 1 bank (uses [:, 0:K])

            def rank_chunk(c):
                rk = spool.tile([128, 1], fp32, tag=f"rank{c}")
                if b_eng[c] == "act":
                    scr = scpool.tile([128, S], fp32, tag="cmp_scr2")
                    rsgn = spool.tile([128, 1], fp32, tag=f"rsgn{c}")
                    nc.scalar.activation(scr[:], sb_bc[:], Act.Sign,
                                         bias=nstok_col(c),
                                         accum_out=rsgn[:])
                    nc.vector.tensor_scalar(rk[:], rsgn[:], 0.5,
                                            1023.5, Alu.mult, Alu.add)
                else:
                    scr = scpool.tile([128, S], fp32, tag="cmp_scr")
                    nc.vector.tensor_scalar(
                        scr[:], sb_bc[:], stok_col(c), 0.0,
                        Alu.is_gt, Alu.add, accum_out=rk[:])
                return rk

            for c in range(NCH):
                rk = rank_chunk(c)
                oh = scpool.tile([128, K], bf16, tag="oh")
                nc.vector.tensor_scalar(oh[:], sb_iota, rk[:], None,
                                        Alu.is_equal)
                nc.tensor.matmul(ps_G[:, 0:K],
                                 sb_fb16[:, c * DA:(c + 1) * DA], oh[:],
                                 start=(c == 0), stop=(c == NCH - 1))

            sb_qT2 = cpool.tile([128, SH], bf16)
            nc.scalar.copy(sb_qT2[:], ps_q[:])
            # selected raw features -> SBUF, then kvb for slot pairs via wkv:
            # evens land in rows 0:64, odds in rows 64:128 (strided moving AP)
            sb_G = spool.tile([17, K], bf16)
            nc.vector.tensor_copy(sb_G[:], ps_G[:, 0:K])
            pK = es_c.enter_context(tc.tile_pool(name="psKvb", bufs=1,
                                                 space="PSUM"))
            ps_kvb = pK.tile([128, KP], fp32)
            nc.tensor.matmul(ps_kvb[0:H1, :], sb_wkv, sb_G[:, 0:KP],
                             start=True, stop=True)
            nc.tensor.matmul(ps_kvb[H1:128, :], sb_wkv, sb_G[:, KP:K],
                             start=True, stop=True)
            sb_kvb2a = spool.tile([128, 51], fp32)
            nc.vector.tensor_copy(sb_kvb2a[:], ps_kvb[:, 0:51])
            sb_kvb2b = spool.tile([128, KP - 51], fp32)
            nc.vector.tensor_copy(sb_kvb2b[:], ps_kvb[:, 51:KP])
            es_c.close()

            def kvb_col(p):
                return (sb_kvb2a[:, p:p + 1] if p < 51
                        else sb_kvb2b[:, p - 51:p - 50])

            pmain = es.enter_context(
                tc.tile_pool(name="main_psum", bufs=3, space="PSUM"))
            pout = es.enter_context(
                tc.tile_pool(name="out_psum", bufs=1, space="PSUM"))

            # ---- stage E: main pairwise loop over 51 groups of 4 tokens.
            # h1 producers are emitted LOOK groups ahead of their matmuls so
            # slow ACT/Pool h1 tiles never stall the PE stream. ----
            ps_out = pout.tile([128, SH], fp32, tag="outacc")   # 2 banks
            i8 = [0]
            h1_tiles = {}
            h2_tiles = {}
            first_w4 = [True]

            def h1_write(dst, p, eng):
                if eng == "act":
                    nc.scalar.activation(dst, sb_qT2[:], Act.Relu,
                                         bias=kvb_col(p))
                else:
                    nc.vector.tensor_scalar(dst, sb_qT2[:], kvb_col(p), 0.0,
                                            Alu.add, Alu.max)

            # ACT-assigned bf16 h1 pairs are emitted UPFRONT: they slot into
            # ACT's idle window between its rank chunks and the h2 stream,
            # instead of convoying the PE mid-loop.
            pre_tiles = {}
            for p in range(KP):
                if h1_eng[p] == "act" and p // 2 not in fp8_groups:
                    h1p = h1pool.tile([128, SH], bf16, tag="h1pre",
                                      name=f"h1pre{p}")
                    nc.scalar.activation(h1p[:], sb_qT2[:], Act.Relu,
                                         bias=kvb_col(p))
                    pre_tiles[p] = h1p

            def produce_h1(g):
                if g in fp8_groups:
                    h18 = h18pool.tile([128, 2, SH], f8, tag="h18")
                    for half in range(2):
                        h1_write(h18[:, half, :], 2 * g + half, eng8[i8[0]])
                        i8[0] += 1
                    h1_tiles[g] = h18
                else:
                    ts = []
                    for half in range(2):
                        p = 2 * g + half
                        if p in pre_tiles:
                            ts.append(pre_tiles.pop(p))
                            continue
                        h1t = h1pool.tile([128, SH], bf16, tag="h1")
                        h1_write(h1t[:], p, h1_eng[p])
                        ts.append(h1t)
                    h1_tiles[g] = ts

            def consume_group(g):
                ps_h = pmain.tile([128, SH], fp32, tag="hps")
                src = h1_tiles.pop(g)
                if g in fp8_groups:
                    for qc in range(2):
                        sl = slice(qc * 512, (qc + 1) * 512)
                        nc.tensor.matmul(ps_h[:, sl], sb_w28,
                                         src[:, :, sl], start=True, stop=True,
                                         perf_mode=DR)
                else:
                    for half in range(2):
                        rows = slice(half * H1, (half + 1) * H1)
                        for n in range(2):
                            sl = slice(n * 512, (n + 1) * 512)
                            nc.tensor.matmul(ps_h[rows, sl], sb_bdmw2,
                                             src[half][:, sl],
                                             start=True, stop=True)
                # h2: relu+bias -> fp8 into paired super-tile
                if g % 2 == 0:
                    h2_tiles[g] = h2pool.tile([128, 2, SH], f8, tag="h2d",
                                              name=f"h2d{g}")
                h2cur = h2_tiles[g - g % 2]
                dst = h2cur[:, g % 2, :]
                eng = h2_eng[g]
                if eng == "act":
                    nc.scalar.activation(dst, ps_h[:], Act.Relu, bias=sb_mb24)
                else:
                    nc.vector.tensor_scalar(dst, ps_h[:], sb_mb24, 0.0,
                                            Alu.add, Alu.max)
                # w4: fp8 DoubleRow over 8 tokens (2 groups)
                if g % 2 == 1:
                    h2cur = h2_tiles.pop(g - 1)
                    for qc in range(2):
                        sl = slice(qc * 512, (qc + 1) * 512)
                        nc.tensor.matmul(ps_out[:, sl], sb_w348,
                                         h2cur[:, :, sl],
                                         start=first_w4[0],
                                         stop=False, skip_group_check=True,
                                         perf_mode=DR)
                    first_w4[0] = False
                elif g == NG - 1:   # lone tail group: plain fp8 matmul
                    h2cur = h2_tiles.pop(g)
                    for qc in range(2):
                        sl = slice(qc * 512, (qc + 1) * 512)
                        nc.tensor.matmul(ps_out[:, sl], sb_w348[:, 0, :],
                                         h2cur[:, 0, sl],
                                         start=first_w4[0],
                                         stop=(qc == 1),
                                         skip_group_check=True)
                    first_w4[0] = False

            for gi in range(NG + LOOK):
                if gi < NG:
                    produce_h1(gi)
                if gi >= LOOK:
                    consume_group(gi - LOOK)

            # ---- stage F: scale + bias + store (host adds resid rows) ----
            sb_out = spool.tile([128, SH], fp32)
            nc.scalar.activation(sb_out[0:32, :], ps_out[0:32, :],
                                 Act.Identity, bias=sb_mb3x[0:32, :],
                                 scale=INV_K)
            nc.sync.dma_start(d_outT[:], sb_out[0:32, :])

    nc.compile()
    return nc


def _host_inputs(full, sw1, sb1, sw2, sb2, mw1, mb1, mw2, mb2, mw3, mb3):
    """Build the 8 per-core input maps (host-side sharding + layout prep)."""
    import ml_dtypes
    f32 = np.float32
    bf = ml_dtypes.bfloat16
    f8 = ml_dtypes.float8_e4m3

    full = np.asarray(full, dtype=f32)
    ones_row = np.ones((1, S), dtype=f32)

    c16 = np.zeros((128, C16_W), dtype=f32)
    c16[:, 0:KP] = np.arange(0, K, 2, dtype=f32)[None, :]
    c16[:, KP:K] = np.arange(1, K, 2, dtype=f32)[None, :]
    c16[0:DA, C16_SW1:C16_SW1 + H2] = np.concatenate(
        [np.asarray(sw1, f32), np.asarray(sb1, f32)[None, :]], axis=0)
    c16[0:H2, C16_SW2] = np.asarray(sw2, f32).reshape(H2)
    c16[0:DA, C16_WKV:C16_WKV + H1] = np.concatenate(
        [np.asarray(mw1[D:2 * D] + mw1[2 * D:], f32),
         np.asarray(mb1, f32)[None, :]], axis=0)
    c16[0:D, C16_WQ:C16_WQ + H1] = np.asarray(mw1[:D], f32)
    bd = np.zeros((128, H1), dtype=f32)
    bd[0:H1, 0:H2] = mw2
    bd[H1:128, H2:H1] = mw2
    c16[:, C16_BDMW2:C16_BDMW2 + H1] = bd
    c16 = c16.astype(bf)

    c32 = np.zeros((128, C32_W), dtype=f32)
    c32[:, C32_MB24] = np.tile(np.asarray(mb2, f32), 4)
    c32[0:D, C32_MB3X] = np.asarray(mb3, f32)
    c32[0, C32_ONE] = 1.0

    # fp8 weights: mw2 DoubleRow block + [q8(mw3); resid] DoubleRow block
    mw2_8 = np.asarray(mw2, f32).astype(f8)
    w28 = np.zeros((128, 2, 128), dtype=f8)
    w28[0:H1, 0, 0:32] = mw2_8
    w28[H1:128, 0, 32:64] = mw2_8
    w28[0:H1, 1, 64:96] = mw2_8
    w28[H1:128, 1, 96:128] = mw2_8
    mw3_8 = np.asarray(mw3, f32).astype(f8)
    mw3_r = (np.asarray(mw3, f32) - mw3_8.astype(f32)).astype(f8)
    w348 = np.zeros((128, 2, 128), dtype=f8)
    for t in range(2):
        w348[:, t, 0:16] = np.tile(mw3_8, (4, 1))
        w348[:, t, 16:32] = np.tile(mw3_r, (4, 1))
    c8 = np.concatenate([w28, w348], axis=2)

    shared = dict(c16=c16, c32=c32, c8=c8)
    in_maps = []
    for c in range(N_CORES):
        b, h = c // 2, c % 2
        fbT = np.concatenate(
            [np.ascontiguousarray(full[b].T), ones_row], axis=0)
        # token-major chunks with ones column: fb16[:, 17c:17c+17]
        fb = np.concatenate([full[b], np.ones((S, 1), np.float32)], axis=1)
        fb16 = np.ascontiguousarray(
            fb.reshape(NCH, 128, DA).transpose(1, 0, 2).reshape(128, NCH * DA))
        m = dict(shared)
        m["fbT16"] = fbT.astype(bf)
        m["fb16"] = fb16.astype(bf)
        m["fqT16"] = np.ascontiguousarray(
            full[b, h * SH:(h + 1) * SH, :].T).astype(bf)
        in_maps.append(m)
    return in_maps


def get_module():
    if "nc" not in _cache:
        _cache["nc"] = _build_module()
    return _cache["nc"]


def run_cores(in_maps):
    from concourse.bass_utils import run_bass_kernel_spmd
    nc = get_module()
    return run_bass_kernel_spmd(nc, in_maps, list(range(N_CORES))).results


def kernel(full, sw1, sb1, sw2, sb2, mw1, mb1, mw2, mb2, mw3, mb3):
    in_maps = _host_inputs(full, sw1, sb1, sw2, sb2, mw1, mb1, mw2, mb2,
                           mw3, mb3)
    results = run_cores(in_maps)
    out = np.empty((B, S, D), dtype=np.float32)
    for c in range(N_CORES):
        b, h = c // 2, c % 2
        oT = results[c]["outT"].astype(np.float32)
        out[b, h * SH:(h + 1) * SH, :] = (oT[0:D] + oT[D:2 * D]).T
    return out
